# revision 8
# baseline (speedup 1.0000x reference)
"""Trainium2 Bass kernel for nn_DictMoEDirect (moe_routing).

Reference computation (fp32, shapes hardcoded):
  x = hidden_states.transpose(1,0,2)              # [B,S,H]
  g = mean_s(relu(x@gW1.T + gb1) @ gW2.T + gb2)   # [B,E]
  W1_b = sum_e g[b,e] eW1[e]; b1_b = g[b]@eb1     # per-sample merged MLP
  W2_b = sum_e g[b,e] eW2[e]; b2_b = g[b]@eb2
  y = relu(x@W1_b.T + b1_b) @ W2_b.T + b2_b       # [B,S,H]
  return y.transpose(1,0,2)                       # [S,B,H]

Distribution (8 NeuronCores, hint: data-parallel/expert-parallel hybrid):
  - Gate: data-parallel (core b computes g[b] from its own sample),
    then a tiny AllGather of g (256 B).
  - Expert FFN: tensor-parallel over DFF. Core j owns DFF slice j of 512.
    It merges its slice of W1/W2 for ALL samples (identity-scaled-matmul
    trick on the PE: lhsT = diag(g[b,e]) accumulated over e in PSUM) and
    computes y1[:, dff_j] for all samples, then the layer-2 partial
    products, which are summed across cores with two ReduceScatters
    (one per H-half, so the first overlaps the second half's compute).
  All matmuls run in float32r (TF32) at 1 cycle/row with fp32 PSUM
  accumulation.

kernel(**inputs) takes full unsharded inputs and returns the full
[S,B,H] output. Self-contained: hardcodes all shapes.
"""

import numpy as np

import concourse.bass as bass
import concourse.mybir as mybir
from concourse import bacc
from concourse.tile import TileContext
from concourse.masks import make_identity

H = 1024
DFF = 4096
E = 8
B = 8
S = 512
NC = 8
DSL = DFF // NC  # 512, per-core DFF slice
P = 128
F32 = mybir.dt.float32
F32R = mybir.dt.float32r
AF = mybir.ActivationFunctionType


def build_module(debug=False):
    nc = bacc.Bacc()

    # ---- I/O ----
    xt_all = nc.declare_dram_parameter("xt_all", [B, H, S], F32, isOutput=False)
    xt_own = nc.declare_dram_parameter("xt_own", [H, S], F32, isOutput=False)
    gw1t = nc.declare_dram_parameter("gw1t", [H, H], F32, isOutput=False)
    gb1t = nc.declare_dram_parameter("gb1t", [P, 8], F32, isOutput=False)
    gw2t = nc.declare_dram_parameter("gw2t", [H, E], F32, isOutput=False)
    gb2 = nc.declare_dram_parameter("gb2", [E], F32, isOutput=False)
    ew1t = nc.declare_dram_parameter("ew1t", [E, H, DSL], F32, isOutput=False)
    ew2t = nc.declare_dram_parameter("ew2t", [E, DSL, H], F32, isOutput=False)
    eb1s = nc.declare_dram_parameter("eb1s", [E, DSL], F32, isOutput=False)
    eb2 = nc.declare_dram_parameter("eb2", [E, H], F32, isOutput=False)
    y_out = nc.declare_dram_parameter("y2t", [H, S], F32, isOutput=True)
    if debug:
        dbg_g = nc.declare_dram_parameter("dbg_g", [NC * E], F32, isOutput=True)
        dbg_y1 = nc.declare_dram_parameter("dbg_y1", [4, P, S], F32, isOutput=True)
        dbg_w1t = nc.declare_dram_parameter("dbg_w1t", [P, 8 * 256], F32, isOutput=True)
        dbg_rs0 = nc.declare_dram_parameter("dbg_rs0", [4, P, S], F32, isOutput=True)

    # ---- internal DRAM ----
    ag_in = nc.dram_tensor("ag_in", [E], F32)
    ag_out = nc.dram_tensor("ag_out", [NC * E], F32, addr_space="Shared")
    y1_dram = nc.dram_tensor("y1_dram", [B, 4, P, S], F32R)
    rs_in0 = nc.dram_tensor("rs_in0", [B, 4, P, S], F32)
    rs_in1 = nc.dram_tensor("rs_in1", [B, 4, P, S], F32)
    rs_out0 = nc.dram_tensor("rs_out0", [4 * P, S], F32)
    rs_out1 = nc.dram_tensor("rs_out1", [4 * P, S], F32)
    groups = [list(range(NC))]

    with TileContext(nc) as tc:
        with (
            tc.tile_pool(name="main", bufs=1) as pool,
            tc.tile_pool(name="psum", bufs=2, space="PSUM") as pp,
        ):
            # ---------------- gate (own sample) ----------------
            xo = pool.tile([P, 8 * S], F32R, tag="xb", bufs=3)
            nc.gpsimd.dma_start(
                xo[:].rearrange("p (k s) -> p k s", k=8),
                xt_own.rearrange("(k p) s -> p k s", p=P),
            )
            gb1_sb = pool.tile([P, 8], F32, tag="gb1")
            nc.sync.dma_start(gb1_sb[:], gb1t[:])
            h1 = pool.tile([P, 8 * S], F32R, tag="h1")
            with tc.tile_pool(name="gatew", bufs=1) as gwpool:
                gw1_r = gwpool.tile([P, 8 * H], F32R, tag="gw")
                for k in range(8):
                    nc.gpsimd.dma_start(
                        gw1_r[:, k * H : (k + 1) * H],
                        gw1t[k * P : (k + 1) * P, :],
                    )
                for m in range(8):
                    ps = pp.tile([P, S], F32, tag="out")
                    for k in range(8):
                        nc.tensor.matmul(
                            ps[:],
                            gw1_r[:, k * H + m * P : k * H + (m + 1) * P],
                            xo[:, k * S : (k + 1) * S],
                            start=(k == 0),
                            stop=(k == 7),
                        )
                    nc.scalar.activation(
                        h1[:, m * S : (m + 1) * S],
                        ps[:],
                        AF.Relu,
                        bias=gb1_sb[:, m : m + 1],
                    )
                gw2_r = gwpool.tile([P, 64], F32R, tag="gw2")
                for k in range(8):
                    nc.gpsimd.dma_start(
                        gw2_r[:, k * E : (k + 1) * E],
                        gw2t[k * P : (k + 1) * P, :],
                    )
                ps_g = pp.tile([E, S], F32, tag="tiny")
                for k in range(8):
                    nc.tensor.matmul(
                        ps_g[:],
                        gw2_r[:, k * E : (k + 1) * E],
                        h1[:, k * S : (k + 1) * S],
                        start=(k == 0),
                        stop=(k == 7),
                    )
                gsum = pool.tile([E, 1], F32, tag="gsum")
                nc.vector.reduce_sum(gsum[:], ps_g[:], axis=mybir.AxisListType.X)
                gb2_sb = pool.tile([E, 1], F32, tag="gb2")
                nc.sync.dma_start(gb2_sb[:], gb2[:, None])
                gmean = pool.tile([E, 1], F32, tag="gmean")
                nc.vector.tensor_scalar_mul(gmean[:], gsum[:], 1.0 / S)
                gown = pool.tile([E, 1], F32, tag="gown")
                nc.vector.tensor_add(gown[:], gmean[:], gb2_sb[:])
                nc.sync.dma_start(ag_in[:], gown[:, 0])

            nc.gpsimd.collective_compute(
                "AllGather",
                mybir.AluOpType.bypass,
                ins=[ag_in[:]],
                outs=[ag_out[:]],
                replica_groups=groups,
            )

            if debug:
                nc.sync.dma_start(dbg_g[:], ag_out[:])
            # g broadcast across partitions [P, B*E]; transposed tiny [E, B]
            g_bc = pool.tile([P, NC * E], F32, tag="gbc")
            nc.sync.dma_start(g_bc[:], ag_out.ap()[None, :].broadcast_to([P, NC * E]))
            gT_r = pool.tile([E, B], F32R, tag="gT")
            nc.gpsimd.dma_start(gT_r[:], ag_out.rearrange("(b e) -> e b", e=E))

            # identity for the scaled-diag merge trick
            eye = pool.tile([P, P], F32, tag="eye")
            make_identity(nc, eye[:])

            # ---- merged per-sample biases ----
            # b1T[:, mt*8+b] = (g[b] @ eb1s)[mt-tile]      (full value)
            # b2T[:, m*8+b]  = (g[b] @ eb2)[m-tile] / 8    (1/8: summed by RS)
            eb1_r = pool.tile([E, DSL], F32R, tag="eb1")
            nc.gpsimd.dma_start(eb1_r[:], eb1s[:])
            eb2_f = pool.tile([E, H], F32, tag="eb2f")
            nc.sync.dma_start(eb2_f[:], eb2[:])
            eb2_r8 = pool.tile([E, H], F32R, tag="eb2r")
            nc.scalar.activation(eb2_r8[:], eb2_f[:], AF.Copy, scale=1.0 / NC)
            b1t = pool.tile([P, 4 * B], F32, tag="b1t")
            b2t = pool.tile([P, 8 * B], F32, tag="b2t")
            for mt in range(4):
                ps = pp.tile([P, B], F32, tag="tiny")
                nc.tensor.matmul(
                    ps[:], eb1_r[:, mt * P : (mt + 1) * P], gT_r[:],
                    start=True, stop=True,
                )
                nc.vector.tensor_copy(b1t[:, mt * B : (mt + 1) * B], ps[:])
            for m in range(8):
                ps = pp.tile([P, B], F32, tag="tiny")
                nc.tensor.matmul(
                    ps[:], eb2_r8[:, m * P : (m + 1) * P], gT_r[:],
                    start=True, stop=True,
                )
                nc.vector.tensor_copy(b2t[:, m * B : (m + 1) * B], ps[:])

            def make_gdiag(b):
                tiles = []
                for e in range(E):
                    gd = pool.tile([P, P], F32R, tag="gd", bufs=16)
                    nc.scalar.activation(
                        gd[:], eye[:], AF.Copy,
                        scale=g_bc[:, b * E + e : b * E + e + 1],
                    )
                    tiles.append(gd)
                return tiles

            # ---------------- phase 1: layer 1 (dff-half passes) --------
            HF = 256
            for p in range(2):
                with tc.tile_pool(name=f"ew1_{p}", bufs=1) as wp:
                    ew = wp.tile([P, 8 * E * HF], F32R, tag="ew1")
                    for k in range(8):
                        for e in range(E):
                            nc.gpsimd.dma_start(
                                ew[:, (k * E + e) * HF : (k * E + e + 1) * HF],
                                ew1t[
                                    e, k * P : (k + 1) * P,
                                    p * HF : (p + 1) * HF,
                                ],
                            )

                    state = {}

                    def merge1(b):
                        gd = make_gdiag(b)
                        xb = pool.tile([P, 8 * S], F32R, tag="xb", bufs=3)
                        nc.gpsimd.dma_start(
                            xb[:].rearrange("p (k s) -> p k s", k=8),
                            xt_all.rearrange("b (k p) s -> b p k s", p=P)[b],
                        )
                        w1t = pool.tile([P, 8 * HF], F32R, tag="wmt", bufs=2)
                        for k in range(8):
                            ps = pp.tile([P, HF], F32, tag="mm")
                            for e in range(E):
                                nc.tensor.matmul(
                                    ps[:],
                                    gd[e][:],
                                    ew[:, (k * E + e) * HF : (k * E + e + 1) * HF],
                                    start=(e == 0),
                                    stop=(e == E - 1),
                                )
                            nc.vector.tensor_copy(
                                w1t[:, k * HF : (k + 1) * HF], ps[:]
                            )
                        state[b] = (w1t, xb)

                    def gemm1(b):
                        w1t, xb = state.pop(b)
                        if debug and p == 0 and b == 0:
                            nc.sync.dma_start(dbg_w1t[:], w1t[:].bitcast(F32))
                        for m in range(2):
                            mt = p * 2 + m
                            ps = pp.tile([P, S], F32, tag="out")
                            for k in range(8):
                                nc.tensor.matmul(
                                    ps[:],
                                    w1t[:, k * HF + m * P : k * HF + (m + 1) * P],
                                    xb[:, k * S : (k + 1) * S],
                                    start=(k == 0),
                                    stop=(k == 7),
                                )
                            y1 = pool.tile([P, S], F32R, tag="y1", bufs=4)
                            nc.scalar.activation(
                                y1[:], ps[:], AF.Relu,
                                bias=b1t[:, mt * B + b : mt * B + b + 1],
                            )
                            nc.sync.dma_start(y1_dram[b, mt], y1[:])

                    for b in range(B + 1):
                        if b < B:
                            merge1(b)
                        if b >= 1:
                            gemm1(b - 1)

            # ---------------- phase 2: layer 2 (h-half passes) ----------
            HH = 512
            for p in range(2):
                rs_in = rs_in0 if p == 0 else rs_in1
                with tc.tile_pool(name=f"ew2_{p}", bufs=1) as wp:
                    ew = wp.tile([P, 4 * E * HH], F32R, tag="ew2")
                    for kt in range(4):
                        for e in range(E):
                            nc.gpsimd.dma_start(
                                ew[:, (kt * E + e) * HH : (kt * E + e + 1) * HH],
                                ew2t[
                                    e, kt * P : (kt + 1) * P,
                                    p * HH : (p + 1) * HH,
                                ],
                            )

                    state2 = {}

                    def merge2(b):
                        gd = make_gdiag(b)
                        yb = pool.tile([P, 4 * S], F32R, tag="yb", bufs=2)
                        nc.sync.dma_start(
                            yb[:].rearrange("p (k s) -> p k s", k=4),
                            y1_dram.rearrange("b k p s -> b p k s")[b],
                        )
                        w2t = pool.tile([P, 4 * HH], F32R, tag="wmt", bufs=2)
                        for kt in range(4):
                            ps = pp.tile([P, HH], F32, tag="mm")
                            for e in range(E):
                                nc.tensor.matmul(
                                    ps[:],
                                    gd[e][:],
                                    ew[:, (kt * E + e) * HH : (kt * E + e + 1) * HH],
                                    start=(e == 0),
                                    stop=(e == E - 1),
                                )
                            nc.vector.tensor_copy(
                                w2t[:, kt * HH : (kt + 1) * HH], ps[:]
                            )
                        state2[b] = (w2t, yb)

                    def gemm2(b):
                        w2t, yb = state2.pop(b)
                        for m in range(4):
                            mg = p * 4 + m
                            ps = pp.tile([P, S], F32, tag="out")
                            for kt in range(4):
                                nc.tensor.matmul(
                                    ps[:],
                                    w2t[:, kt * HH + m * P : kt * HH + (m + 1) * P],
                                    yb[:, kt * S : (kt + 1) * S],
                                    start=(kt == 0),
                                    stop=(kt == 3),
                                )
                            y2 = pool.tile([P, S], F32, tag="y2", bufs=4)
                            nc.scalar.activation(
                                y2[:], ps[:], AF.Identity,
                                bias=b2t[:, mg * B + b : mg * B + b + 1],
                            )
                            nc.sync.dma_start(rs_in[b, m], y2[:])

                    for b in range(B + 1):
                        if b < B:
                            merge2(b)
                        if b >= 1:
                            gemm2(b - 1)

                nc.gpsimd.collective_compute(
                    "ReduceScatter",
                    mybir.AluOpType.add,
                    ins=[rs_in.ap().rearrange("b m p s -> (b m p) s")],
                    outs=[(rs_out0 if p == 0 else rs_out1)[:]],
                    replica_groups=groups,
                )

            if debug:
                for mt in range(4):
                    nc.sync.dma_start(dbg_y1[mt], y1_dram[0, mt].bitcast(F32))
                for m in range(4):
                    nc.sync.dma_start(dbg_rs0[m], rs_in0[0, m])
            nc.sync.dma_start(y_out[0 : 4 * P], rs_out0[:])
            nc.sync.dma_start(y_out[4 * P : 8 * P], rs_out1[:])

    nc.compile()
    return nc


def _shard_inputs(hidden_states, gW1, gb1, gW2, gb2, eW1, eb1, eW2, eb2):
    xt_all = np.ascontiguousarray(
        np.asarray(hidden_states, dtype=np.float32).transpose(1, 2, 0)
    )  # [B, H, S]
    gW1t = np.ascontiguousarray(np.asarray(gW1, np.float32).T)
    gb1t = np.ascontiguousarray(
        np.asarray(gb1, np.float32).reshape(8, P).T
    )
    gW2t = np.ascontiguousarray(np.asarray(gW2, np.float32).T)
    gb2 = np.ascontiguousarray(np.asarray(gb2, np.float32))
    eW1 = np.asarray(eW1, np.float32)
    eW2 = np.asarray(eW2, np.float32)
    eb1 = np.asarray(eb1, np.float32)
    eb2 = np.ascontiguousarray(np.asarray(eb2, np.float32))
    in_maps = []
    for j in range(NC):
        sl = slice(j * DSL, (j + 1) * DSL)
        in_maps.append(
            {
                "xt_all": xt_all,
                "xt_own": np.ascontiguousarray(xt_all[j]),
                "gw1t": gW1t,
                "gb1t": gb1t,
                "gw2t": gW2t,
                "gb2": gb2,
                "ew1t": np.ascontiguousarray(
                    eW1[:, sl, :].transpose(0, 2, 1)
                ),
                "ew2t": np.ascontiguousarray(
                    eW2[:, :, sl].transpose(0, 2, 1)
                ),
                "eb1s": np.ascontiguousarray(eb1[:, sl]),
                "eb2": eb2,
            }
        )
    return in_maps


# ---------------- SPMD runner (persistent jit over axon PJRT) -----------

_CACHE = {}


def _build_runner(debug=False):
    import jax
    from jax.sharding import Mesh, PartitionSpec
    from jax.experimental.shard_map import shard_map
    from concourse import bass2jax

    nc = build_module(debug=debug)
    bass2jax.install_neuronx_cc_hook()
    partition_name = nc.partition_id_tensor.name if nc.partition_id_tensor else None

    in_names, out_names, out_avals = [], [], []
    for alloc in nc.m.functions[0].allocations:
        if not isinstance(alloc, mybir.MemoryLocationSet):
            continue
        name = alloc.memorylocations[0].name
        if alloc.kind == "ExternalInput":
            if name != partition_name:
                in_names.append(name)
        elif alloc.kind == "ExternalOutput":
            out_avals.append(
                jax.core.ShapedArray(
                    tuple(alloc.tensor_shape), mybir.dt.np(alloc.dtype)
                )
            )
            out_names.append(name)
    n_outs = len(out_names)
    all_in_names = list(in_names) + list(out_names)
    if partition_name is not None:
        all_in_names.append(partition_name)

    def _body(*args):
        operands = list(args)
        if partition_name is not None:
            operands.append(bass2jax.partition_id_tensor())
        return tuple(
            bass2jax._bass_exec_p.bind(
                *operands,
                out_avals=tuple(out_avals),
                in_names=tuple(all_in_names),
                out_names=tuple(out_names),
                lowering_input_output_aliases=(),
                sim_require_finite=True,
                sim_require_nnan=True,
                nc=nc,
            )
        )

    devices = jax.devices()[:NC]
    mesh = Mesh(np.asarray(devices), ("core",))
    n_params = len(in_names)
    sharded = jax.jit(
        shard_map(
            _body,
            mesh=mesh,
            in_specs=(PartitionSpec("core"),) * (n_params + n_outs),
            out_specs=(PartitionSpec("core"),) * n_outs,
            check_rep=False,
        ),
        keep_unused=True,
    )
    zero_shapes = [((NC * a.shape[0], *a.shape[1:]), a.dtype) for a in out_avals]

    def run(in_maps, device_inputs=None):
        import jax

        if device_inputs is None:
            concat_in = [
                np.concatenate(
                    [np.asarray(in_maps[c][n]) for c in range(NC)], axis=0
                )
                for n in in_names
            ]
            device_inputs = [jax.device_put(x) for x in concat_in]
            jax.block_until_ready(device_inputs)
        concat_zeros = [np.zeros(s, d) for s, d in zero_shapes]
        out_arrs = sharded(*device_inputs, *concat_zeros)
        jax.block_until_ready(out_arrs)
        results = [
            {
                name: np.asarray(out_arrs[i]).reshape(NC, *out_avals[i].shape)[c]
                for i, name in enumerate(out_names)
            }
            for c in range(NC)
        ]
        return results, device_inputs

    return run


def get_runner(debug=False):
    key = ("run", debug)
    if key not in _CACHE:
        _CACHE[key] = _build_runner(debug=debug)
    return _CACHE[key]


def kernel(**inputs) -> np.ndarray:
    run = get_runner()
    in_maps = _shard_inputs(**inputs)
    results, _ = run(in_maps)
    # core b's output is y2^T[b] = [H, S]; assemble [S, B, H]
    y2t = np.stack([results[b]["y2t"] for b in range(B)], axis=0)  # [B, H, S]
    return np.ascontiguousarray(y2t.transpose(2, 0, 1)).astype(np.float32)


# revision 11
# speedup vs baseline: 1.1199x; 1.1199x over previous
"""Trainium2 Bass kernel for nn_DictMoEDirect (moe_routing).

Reference computation (fp32, shapes hardcoded):
  x = hidden_states.transpose(1,0,2)              # [B,S,H]
  g = mean_s(relu(x@gW1.T + gb1) @ gW2.T + gb2)   # [B,E]
  W1_b = sum_e g[b,e] eW1[e]; b1_b = g[b]@eb1     # per-sample merged MLP
  W2_b = sum_e g[b,e] eW2[e]; b2_b = g[b]@eb2
  y = relu(x@W1_b.T + b1_b) @ W2_b.T + b2_b       # [B,S,H]
  return y.transpose(1,0,2)                       # [S,B,H]

Distribution over 8 NeuronCores:
  - Gate: data-parallel (core b computes g[b] from its own sample), then a
    tiny AllGather of g (256 B).
  - Expert FFN: tensor-parallel over DFF. Core j owns DFF slice j (512 wide).
    It merges its slice of W1/W2 for ALL samples with the identity-scaled
    matmul trick on the PE (lhsT = diag(g[b,e]), accumulated over e in PSUM),
    computes y1[:, dff_j] for all samples, then layer-2 partial products,
    summed across cores with two ReduceScatters (one per H-half so the first
    overlaps the second half's compute).
  All matmuls run in float32r (TF32) at 1 cycle/row with fp32 PSUM accum.

kernel(**inputs) takes full unsharded inputs, shards/transposes on the host,
runs the SPMD kernel, and reassembles the full [S,B,H] output.
"""

import numpy as np

import concourse.bass as bass  # noqa: F401
import concourse.mybir as mybir
from concourse import bacc
from concourse.tile import TileContext
from concourse.masks import make_identity

H = 1024
DFF = 4096
E = 8
B = 8
S = 512
NC = 8
DSL = DFF // NC  # 512, per-core DFF slice
P = 128
F32 = mybir.dt.float32
F32R = mybir.dt.float32r
AF = mybir.ActivationFunctionType


def build_module(debug=False, time_loop=0):
    """time_loop=R wraps the FFN phases (not gate/collectives) in an
    on-device For loop for timing runs; outputs are then meaningless."""
    nc = bacc.Bacc()

    # ---- I/O ----
    xt_all = nc.declare_dram_parameter("xt_all", [B, H, S], F32R, isOutput=False)
    xt_own = nc.declare_dram_parameter("xt_own", [H, S], F32R, isOutput=False)
    gw1t = nc.declare_dram_parameter("gw1t", [H, H], F32R, isOutput=False)
    gb1t = nc.declare_dram_parameter("gb1t", [P, 8], F32, isOutput=False)
    gw2t = nc.declare_dram_parameter("gw2t", [H, E], F32R, isOutput=False)
    gb2 = nc.declare_dram_parameter("gb2", [E], F32, isOutput=False)
    ew1d = nc.declare_dram_parameter("ew1d", [2, 8, P, E * 256], F32R, isOutput=False)
    ew2d = nc.declare_dram_parameter("ew2d", [2, 4, P, E * 512], F32R, isOutput=False)
    eb1s = nc.declare_dram_parameter("eb1s", [E, DSL], F32R, isOutput=False)
    eb2 = nc.declare_dram_parameter("eb2", [E, H], F32, isOutput=False)
    y_out = nc.declare_dram_parameter("y2t", [H, S], F32, isOutput=True)
    if debug:
        dbg_g = nc.declare_dram_parameter("dbg_g", [NC * E], F32, isOutput=True)
        dbg_y1 = nc.declare_dram_parameter("dbg_y1", [4, P, S], F32, isOutput=True)
        dbg_w1t = nc.declare_dram_parameter(
            "dbg_w1t", [P, 8 * 256], F32, isOutput=True
        )
        dbg_rs0 = nc.declare_dram_parameter("dbg_rs0", [4, P, S], F32, isOutput=True)

    # ---- internal DRAM ----
    ag_in = nc.dram_tensor("ag_in", [E], F32)
    ag_out = nc.dram_tensor("ag_out", [NC * E], F32, addr_space="Shared")
    y1_dram = nc.dram_tensor("y1_dram", [B, 4, P, S], F32R)
    rs_in0 = nc.dram_tensor("rs_in0", [B, 4, P, S], F32)
    rs_in1 = nc.dram_tensor("rs_in1", [B, 4, P, S], F32)
    rs_out0 = nc.dram_tensor("rs_out0", [4 * P, S], F32)
    rs_out1 = nc.dram_tensor("rs_out1", [4 * P, S], F32)
    groups = [list(range(NC))]

    with TileContext(nc) as tc:
        with (
            tc.tile_pool(name="main", bufs=1) as pool,
            tc.tile_pool(name="psum", bufs=2, space="PSUM") as pp,
        ):
            # ---------------- gate (own sample) ----------------
            xo = pool.tile([P, 8 * S], F32R, tag="xb", bufs=3)
            nc.sync.dma_start(
                xo[:].rearrange("p (k s) -> p k s", k=8),
                xt_own.rearrange("(k p) s -> p k s", p=P),
            )
            gb1_sb = pool.tile([P, 8], F32, tag="gb1")
            nc.sync.dma_start(gb1_sb[:], gb1t[:])
            h1 = pool.tile([P, 8 * S], F32R, tag="h1")
            with tc.tile_pool(name="gatew", bufs=1) as gwpool:
                gw1_r = gwpool.tile([P, 8 * H], F32R, tag="gw")
                for k in range(8):
                    nc.sync.dma_start(
                        gw1_r[:, k * H : (k + 1) * H],
                        gw1t[k * P : (k + 1) * P, :],
                    )
                for m in range(8):
                    ps = pp.tile([P, S], F32, tag="out")
                    for k in range(8):
                        nc.tensor.matmul(
                            ps[:],
                            gw1_r[:, k * H + m * P : k * H + (m + 1) * P],
                            xo[:, k * S : (k + 1) * S],
                            start=(k == 0),
                            stop=(k == 7),
                        )
                    nc.scalar.activation(
                        h1[:, m * S : (m + 1) * S],
                        ps[:],
                        AF.Relu,
                        bias=gb1_sb[:, m : m + 1],
                    )
                gw2_r = gwpool.tile([P, 64], F32R, tag="gw2")
                for k in range(8):
                    nc.sync.dma_start(
                        gw2_r[:, k * E : (k + 1) * E],
                        gw2t[k * P : (k + 1) * P, :],
                    )
                ps_g = pp.tile([E, S], F32, tag="tiny")
                for k in range(8):
                    nc.tensor.matmul(
                        ps_g[:],
                        gw2_r[:, k * E : (k + 1) * E],
                        h1[:, k * S : (k + 1) * S],
                        start=(k == 0),
                        stop=(k == 7),
                    )
                gsum = pool.tile([E, 1], F32, tag="gsum")
                nc.vector.reduce_sum(gsum[:], ps_g[:], axis=mybir.AxisListType.X)
                gb2_sb = pool.tile([E, 1], F32, tag="gb2")
                nc.sync.dma_start(gb2_sb[:], gb2[:, None])
                gmean = pool.tile([E, 1], F32, tag="gmean")
                nc.vector.tensor_scalar_mul(gmean[:], gsum[:], 1.0 / S)
                gown = pool.tile([E, 1], F32, tag="gown")
                nc.vector.tensor_add(gown[:], gmean[:], gb2_sb[:])
                nc.sync.dma_start(ag_in[:], gown[:, 0])

            nc.gpsimd.collective_compute(
                "AllGather",
                mybir.AluOpType.bypass,
                ins=[ag_in[:]],
                outs=[ag_out[:]],
                replica_groups=groups,
            )
            if debug:
                nc.sync.dma_start(dbg_g[:], ag_out[:])

            # g broadcast across partitions [P, B*E]; transposed tiny [E, B]
            g_bc = pool.tile([P, NC * E], F32, tag="gbc")
            nc.sync.dma_start(
                g_bc[:], ag_out.ap()[None, :].broadcast_to([P, NC * E])
            )
            gT_r = pool.tile([E, B], F32R, tag="gT")
            nc.gpsimd.dma_start(gT_r[:], ag_out.rearrange("(b e) -> e b", e=E))

            # identity for the scaled-diag merge trick
            eye = pool.tile([P, P], F32, tag="eye")
            make_identity(nc, eye[:])

            # ---- merged per-sample biases ----
            # b1T[:, mt*8+b] = (g[b] @ eb1s)[mt-tile]      (full value)
            # b2T[:, m*8+b]  = (g[b] @ eb2)[m-tile] / 8    (1/8: summed by RS)
            eb1_r = pool.tile([E, DSL], F32R, tag="eb1")
            nc.sync.dma_start(eb1_r[:], eb1s[:])
            eb2_f = pool.tile([E, H], F32, tag="eb2f")
            nc.sync.dma_start(eb2_f[:], eb2[:])
            eb2_r8 = pool.tile([E, H], F32R, tag="eb2r")
            nc.scalar.activation(eb2_r8[:], eb2_f[:], AF.Copy, scale=1.0 / NC)
            b1t = pool.tile([P, 4 * B], F32, tag="b1t")
            b2t = pool.tile([P, 8 * B], F32, tag="b2t")
            for mt in range(4):
                ps = pp.tile([P, B], F32, tag="tiny")
                nc.tensor.matmul(
                    ps[:],
                    eb1_r[:, mt * P : (mt + 1) * P],
                    gT_r[:],
                    start=True,
                    stop=True,
                )
                nc.vector.tensor_copy(b1t[:, mt * B : (mt + 1) * B], ps[:])
            for m in range(8):
                ps = pp.tile([P, B], F32, tag="tiny")
                nc.tensor.matmul(
                    ps[:],
                    eb2_r8[:, m * P : (m + 1) * P],
                    gT_r[:],
                    start=True,
                    stop=True,
                )
                nc.vector.tensor_copy(b2t[:, m * B : (m + 1) * B], ps[:])

            def make_gdiag(b):
                tiles = []
                for e in range(E):
                    gd = pool.tile([P, P], F32R, tag="gd", bufs=16)
                    nc.scalar.activation(
                        gd[:],
                        eye[:],
                        AF.Copy,
                        scale=g_bc[:, b * E + e : b * E + e + 1],
                    )
                    tiles.append(gd)
                return tiles

            def phase1():
                HF = 256
                for p in range(2):
                    with tc.tile_pool(name=f"ew1_{p}", bufs=1) as wp:
                        ew = wp.tile([P, 8 * E * HF], F32R, tag="ew1")
                        for k in range(8):
                            nc.sync.dma_start(
                                ew[:, k * E * HF : (k + 1) * E * HF],
                                ew1d[p, k],
                            )

                        state = {}

                        def merge1(b):
                            gd = make_gdiag(b)
                            xb = pool.tile([P, 8 * S], F32R, tag="xb", bufs=3)
                            nc.sync.dma_start(
                                xb[:].rearrange("p (k s) -> p k s", k=8),
                                xt_all.rearrange("b (k p) s -> b p k s", p=P)[b],
                            )
                            w1t = pool.tile([P, 8 * HF], F32R, tag="wmt", bufs=2)
                            for k in range(8):
                                ps = pp.tile([P, HF], F32, tag="mm")
                                for e in range(E):
                                    nc.tensor.matmul(
                                        ps[:],
                                        gd[e][:],
                                        ew[
                                            :,
                                            (k * E + e) * HF : (k * E + e + 1) * HF,
                                        ],
                                        start=(e == 0),
                                        stop=(e == E - 1),
                                    )
                                nc.vector.tensor_copy(
                                    w1t[:, k * HF : (k + 1) * HF], ps[:]
                                )
                            state[b] = (w1t, xb)

                        def gemm1(b):
                            w1t, xb = state.pop(b)
                            if debug and p == 0 and b == 0:
                                nc.sync.dma_start(dbg_w1t[:], w1t[:].bitcast(F32))
                            for m in range(2):
                                mt = p * 2 + m
                                ps = pp.tile([P, S], F32, tag="out")
                                for k in range(8):
                                    nc.tensor.matmul(
                                        ps[:],
                                        w1t[
                                            :, k * HF + m * P : k * HF + (m + 1) * P
                                        ],
                                        xb[:, k * S : (k + 1) * S],
                                        start=(k == 0),
                                        stop=(k == 7),
                                    )
                                y1 = pool.tile([P, S], F32R, tag="y1", bufs=4)
                                nc.scalar.activation(
                                    y1[:],
                                    ps[:],
                                    AF.Relu,
                                    bias=b1t[:, mt * B + b : mt * B + b + 1],
                                )
                                nc.sync.dma_start(y1_dram[b, mt], y1[:])

                        for b in range(B + 1):
                            if b < B:
                                merge1(b)
                            if b >= 1:
                                gemm1(b - 1)

            def phase2(with_rs=True):
                HH = 512
                for p in range(2):
                    rs_in = rs_in0 if p == 0 else rs_in1
                    with tc.tile_pool(name=f"ew2_{p}", bufs=1) as wp:
                        ew = wp.tile([P, 4 * E * HH], F32R, tag="ew2")
                        for kt in range(4):
                            nc.sync.dma_start(
                                ew[:, kt * E * HH : (kt + 1) * E * HH],
                                ew2d[p, kt],
                            )

                        state2 = {}

                        def merge2(b):
                            gd = make_gdiag(b)
                            yb = pool.tile([P, 4 * S], F32R, tag="yb", bufs=2)
                            nc.sync.dma_start(
                                yb[:].rearrange("p (k s) -> p k s", k=4),
                                y1_dram.rearrange("b k p s -> b p k s")[b],
                            )
                            w2t = pool.tile([P, 4 * HH], F32R, tag="wmt", bufs=2)
                            for kt in range(4):
                                ps = pp.tile([P, HH], F32, tag="mm")
                                for e in range(E):
                                    nc.tensor.matmul(
                                        ps[:],
                                        gd[e][:],
                                        ew[
                                            :,
                                            (kt * E + e)
                                            * HH : (kt * E + e + 1)
                                            * HH,
                                        ],
                                        start=(e == 0),
                                        stop=(e == E - 1),
                                    )
                                nc.vector.tensor_copy(
                                    w2t[:, kt * HH : (kt + 1) * HH], ps[:]
                                )
                            state2[b] = (w2t, yb)

                        def gemm2(b):
                            w2t, yb = state2.pop(b)
                            for m in range(4):
                                mg = p * 4 + m
                                ps = pp.tile([P, S], F32, tag="out")
                                for kt in range(4):
                                    nc.tensor.matmul(
                                        ps[:],
                                        w2t[
                                            :,
                                            kt * HH + m * P : kt * HH + (m + 1) * P,
                                        ],
                                        yb[:, kt * S : (kt + 1) * S],
                                        start=(kt == 0),
                                        stop=(kt == 3),
                                    )
                                y2 = pool.tile([P, S], F32, tag="y2", bufs=4)
                                nc.scalar.activation(
                                    y2[:],
                                    ps[:],
                                    AF.Identity,
                                    bias=b2t[:, mg * B + b : mg * B + b + 1],
                                )
                                nc.sync.dma_start(rs_in[b, m], y2[:])

                        for b in range(B + 1):
                            if b < B:
                                merge2(b)
                            if b >= 1:
                                gemm2(b - 1)

                    if with_rs:
                        nc.gpsimd.collective_compute(
                            "ReduceScatter",
                            mybir.AluOpType.add,
                            ins=[rs_in.ap().rearrange("b m p s -> (b m p) s")],
                            outs=[(rs_out0 if p == 0 else rs_out1)[:]],
                            replica_groups=groups,
                        )

            if time_loop:
                with tc.For_i(0, time_loop, 1):
                    phase1()
                    phase2(with_rs=False)
                nc.sync.dma_start(y_out[0 : 4 * P], rs_in0.ap()[0])
                nc.sync.dma_start(y_out[4 * P : 8 * P], rs_in1.ap()[0])
            else:
                phase1()
                phase2(with_rs=True)
                if debug:
                    for mt in range(4):
                        nc.sync.dma_start(dbg_y1[mt], y1_dram[0, mt].bitcast(F32))
                    for m in range(4):
                        nc.sync.dma_start(dbg_rs0[m], rs_in0[0, m])
                nc.sync.dma_start(y_out[0 : 4 * P], rs_out0[:])
                nc.sync.dma_start(y_out[4 * P : 8 * P], rs_out1[:])

    nc.compile()
    return nc


def _ew1_dev(a):
    # a: [E, DSL(o), H(i)] -> [2pass, 8k, 128p(i), 8e * 256o]
    a2 = np.ascontiguousarray(np.asarray(a, np.float32).transpose(2, 0, 1))
    a3 = a2.reshape(8, P, E, 2, 256).transpose(3, 0, 1, 2, 4)
    return np.ascontiguousarray(a3.reshape(2, 8, P, E * 256))


def _ew2_dev(c):
    # c: [E, H(h), DSL(d)] -> [2pass, 4kt, 128p(d), 8e * 512h]
    c2 = np.ascontiguousarray(np.asarray(c, np.float32).transpose(2, 0, 1))
    c3 = c2.reshape(4, P, E, 2, 512).transpose(3, 0, 1, 2, 4)
    return np.ascontiguousarray(c3.reshape(2, 4, P, E * 512))


def _shard_inputs(hidden_states, gW1, gb1, gW2, gb2, eW1, eb1, eW2, eb2):
    xt_all = np.ascontiguousarray(
        np.asarray(hidden_states, dtype=np.float32).transpose(1, 2, 0)
    )  # [B, H, S]
    gW1t = np.ascontiguousarray(np.asarray(gW1, np.float32).T)
    gb1t = np.ascontiguousarray(np.asarray(gb1, np.float32).reshape(8, P).T)
    gW2t = np.ascontiguousarray(np.asarray(gW2, np.float32).T)
    gb2 = np.ascontiguousarray(np.asarray(gb2, np.float32))
    eW1 = np.asarray(eW1, np.float32)
    eW2 = np.asarray(eW2, np.float32)
    eb1 = np.asarray(eb1, np.float32)
    eb2 = np.ascontiguousarray(np.asarray(eb2, np.float32))
    in_maps = []
    for j in range(NC):
        sl = slice(j * DSL, (j + 1) * DSL)
        in_maps.append(
            {
                "xt_all": xt_all,
                "xt_own": np.ascontiguousarray(xt_all[j]),
                "gw1t": gW1t,
                "gb1t": gb1t,
                "gw2t": gW2t,
                "gb2": gb2,
                "ew1d": _ew1_dev(eW1[:, sl, :]),
                "ew2d": _ew2_dev(eW2[:, :, sl]),
                "eb1s": np.ascontiguousarray(eb1[:, sl]),
                "eb2": eb2,
            }
        )
    return in_maps


# ---------------- SPMD runner (persistent jit over axon PJRT) -----------

_CACHE = {}


def _build_runner(debug=False, time_loop=0):
    import jax
    from jax.sharding import Mesh, PartitionSpec
    from jax.experimental.shard_map import shard_map
    from concourse import bass2jax

    nc = build_module(debug=debug, time_loop=time_loop)
    bass2jax.install_neuronx_cc_hook()
    partition_name = nc.partition_id_tensor.name if nc.partition_id_tensor else None

    in_names, out_names, out_avals = [], [], []
    for alloc in nc.m.functions[0].allocations:
        if not isinstance(alloc, mybir.MemoryLocationSet):
            continue
        name = alloc.memorylocations[0].name
        if alloc.kind == "ExternalInput":
            if name != partition_name:
                in_names.append(name)
        elif alloc.kind == "ExternalOutput":
            out_avals.append(
                jax.core.ShapedArray(
                    tuple(alloc.tensor_shape), mybir.dt.np(alloc.dtype)
                )
            )
            out_names.append(name)
    n_outs = len(out_names)
    all_in_names = list(in_names) + list(out_names)
    if partition_name is not None:
        all_in_names.append(partition_name)

    def _body(*args):
        operands = list(args)
        if partition_name is not None:
            operands.append(bass2jax.partition_id_tensor())
        return tuple(
            bass2jax._bass_exec_p.bind(
                *operands,
                out_avals=tuple(out_avals),
                in_names=tuple(all_in_names),
                out_names=tuple(out_names),
                lowering_input_output_aliases=(),
                sim_require_finite=True,
                sim_require_nnan=True,
                nc=nc,
            )
        )

    devices = jax.devices()[:NC]
    mesh = Mesh(np.asarray(devices), ("core",))
    n_params = len(in_names)
    sharded = jax.jit(
        shard_map(
            _body,
            mesh=mesh,
            in_specs=(PartitionSpec("core"),) * (n_params + n_outs),
            out_specs=(PartitionSpec("core"),) * n_outs,
            check_rep=False,
        ),
        keep_unused=True,
    )
    zero_shapes = [((NC * a.shape[0], *a.shape[1:]), a.dtype) for a in out_avals]

    def run(in_maps, device_inputs=None, fetch=True):
        if device_inputs is None:
            concat_in = [
                np.concatenate(
                    [np.asarray(in_maps[c][n]) for c in range(NC)], axis=0
                )
                for n in in_names
            ]
            dev_params = [jax.device_put(x) for x in concat_in]
            dev_zeros = [jax.device_put(np.zeros(s, d)) for s, d in zero_shapes]
            device_inputs = (dev_params, dev_zeros)
            jax.block_until_ready(dev_params)
            jax.block_until_ready(dev_zeros)
        dev_params, dev_zeros = device_inputs
        out_arrs = sharded(*dev_params, *dev_zeros)
        jax.block_until_ready(out_arrs)
        if not fetch:
            return None, device_inputs
        results = [
            {
                name: np.asarray(out_arrs[i]).reshape(NC, *out_avals[i].shape)[c]
                for i, name in enumerate(out_names)
            }
            for c in range(NC)
        ]
        return results, device_inputs

    return run


def get_runner(debug=False, time_loop=0):
    key = ("run", debug, time_loop)
    if key not in _CACHE:
        _CACHE[key] = _build_runner(debug=debug, time_loop=time_loop)
    return _CACHE[key]


def kernel(**inputs) -> np.ndarray:
    run = get_runner()
    in_maps = _shard_inputs(**inputs)
    results, _ = run(in_maps)
    # core b's output is y2^T[b] = [H, S]; assemble [S, B, H]
    y2t = np.stack([results[b]["y2t"] for b in range(B)], axis=0)  # [B, H, S]
    return np.ascontiguousarray(y2t.transpose(2, 0, 1)).astype(np.float32)


# revision 12
# speedup vs baseline: 1.2506x; 1.1167x over previous
"""Trainium2 Bass kernel for nn_DictMoEDirect (moe_routing).

Reference computation (fp32, shapes hardcoded):
  x = hidden_states.transpose(1,0,2)              # [B,S,H]
  g = mean_s(relu(x@gW1.T + gb1) @ gW2.T + gb2)   # [B,E]
  W1_b = sum_e g[b,e] eW1[e]; b1_b = g[b]@eb1     # per-sample merged MLP
  W2_b = sum_e g[b,e] eW2[e]; b2_b = g[b]@eb2
  y = relu(x@W1_b.T + b1_b) @ W2_b.T + b2_b       # [B,S,H]
  return y.transpose(1,0,2)                       # [S,B,H]

Distribution over 8 NeuronCores:
  - Gate: data-parallel (core b computes g[b] from its own sample), then a
    tiny AllGather of g (256 B).
  - Expert FFN: tensor-parallel over DFF. Core j owns DFF slice j (512 wide).
    It merges its slice of W1/W2 for ALL samples with the identity-scaled
    matmul trick on the PE (lhsT = diag(g[b,e]), accumulated over e in PSUM),
    computes y1[:, dff_j] for all samples, then layer-2 partial products,
    summed across cores with two ReduceScatters (one per H-half so the first
    overlaps the second half's compute).
  All matmuls run in float32r (TF32) at 1 cycle/row with fp32 PSUM accum.

kernel(**inputs) takes full unsharded inputs, shards/transposes on the host,
runs the SPMD kernel, and reassembles the full [S,B,H] output.
"""

import numpy as np

import concourse.bass as bass  # noqa: F401
import concourse.mybir as mybir
from concourse import bacc
from concourse.tile import TileContext
from concourse.masks import make_identity

H = 1024
DFF = 4096
E = 8
B = 8
S = 512
NC = 8
DSL = DFF // NC  # 512, per-core DFF slice
P = 128
F32 = mybir.dt.float32
F32R = mybir.dt.float32r
AF = mybir.ActivationFunctionType


def build_module(debug=False, time_loop=0):
    """time_loop=R wraps the FFN phases (not gate/collectives) in an
    on-device For loop for timing runs; outputs are then meaningless."""
    nc = bacc.Bacc()

    # ---- I/O ----
    xt_all = nc.declare_dram_parameter("xt_all", [B, H, S], F32R, isOutput=False)
    xt_own = nc.declare_dram_parameter("xt_own", [H, S], F32R, isOutput=False)
    gw1t = nc.declare_dram_parameter("gw1t", [H, H], F32R, isOutput=False)
    gb1t = nc.declare_dram_parameter("gb1t", [P, 8], F32, isOutput=False)
    gw2t = nc.declare_dram_parameter("gw2t", [H, E], F32R, isOutput=False)
    gb2 = nc.declare_dram_parameter("gb2", [E], F32, isOutput=False)
    ew1d = nc.declare_dram_parameter("ew1d", [2, 8, P, E * 256], F32R, isOutput=False)
    ew2d = nc.declare_dram_parameter("ew2d", [2, 4, P, E * 512], F32R, isOutput=False)
    eb1s = nc.declare_dram_parameter("eb1s", [E, DSL], F32R, isOutput=False)
    eb2 = nc.declare_dram_parameter("eb2", [E, H], F32, isOutput=False)
    y_out = nc.declare_dram_parameter("y2t", [H, S], F32, isOutput=True)
    if debug:
        dbg_g = nc.declare_dram_parameter("dbg_g", [NC * E], F32, isOutput=True)
        dbg_y1 = nc.declare_dram_parameter("dbg_y1", [4, P, S], F32, isOutput=True)
        dbg_w1t = nc.declare_dram_parameter(
            "dbg_w1t", [P, 8 * 256], F32, isOutput=True
        )
        dbg_rs0 = nc.declare_dram_parameter("dbg_rs0", [4, P, S], F32, isOutput=True)

    # ---- internal DRAM ----
    ag_in = nc.dram_tensor("ag_in", [E], F32)
    ag_out = nc.dram_tensor("ag_out", [NC * E], F32, addr_space="Shared")
    y1_dram = nc.dram_tensor("y1_dram", [B, 4, P, S], F32R)
    rs_in0 = nc.dram_tensor("rs_in0", [B, 4, P, S], F32)
    rs_in1 = nc.dram_tensor("rs_in1", [B, 4, P, S], F32)
    rs_out0 = nc.dram_tensor("rs_out0", [4 * P, S], F32)
    rs_out1 = nc.dram_tensor("rs_out1", [4 * P, S], F32)
    groups = [list(range(NC))]

    with TileContext(nc) as tc:
        with (
            tc.tile_pool(name="main", bufs=1) as pool,
            tc.tile_pool(name="psum", bufs=2, space="PSUM") as pp,
        ):
            # ---------------- gate (own sample) ----------------
            xo = pool.tile([P, 8 * S], F32R, tag="xb", bufs=2)
            nc.sync.dma_start(
                xo[:].rearrange("p (k s) -> p k s", k=8),
                xt_own.rearrange("(k p) s -> p k s", p=P),
            )
            gb1_sb = pool.tile([P, 8], F32, tag="gb1")
            nc.sync.dma_start(gb1_sb[:], gb1t[:])
            h1 = pool.tile([P, 8 * S], F32R, tag="h1")
            with tc.tile_pool(name="gatew", bufs=1) as gwpool:
                gw1_r = gwpool.tile([P, 8 * H], F32R, tag="gw")
                for k in range(8):
                    nc.sync.dma_start(
                        gw1_r[:, k * H : (k + 1) * H],
                        gw1t[k * P : (k + 1) * P, :],
                    )
                for m in range(8):
                    ps = pp.tile([P, S], F32, tag="out")
                    for k in range(8):
                        nc.tensor.matmul(
                            ps[:],
                            gw1_r[:, k * H + m * P : k * H + (m + 1) * P],
                            xo[:, k * S : (k + 1) * S],
                            start=(k == 0),
                            stop=(k == 7),
                        )
                    nc.scalar.activation(
                        h1[:, m * S : (m + 1) * S],
                        ps[:],
                        AF.Relu,
                        bias=gb1_sb[:, m : m + 1],
                    )
                gw2_r = gwpool.tile([P, 64], F32R, tag="gw2")
                for k in range(8):
                    nc.sync.dma_start(
                        gw2_r[:, k * E : (k + 1) * E],
                        gw2t[k * P : (k + 1) * P, :],
                    )
                ps_g = pp.tile([E, S], F32, tag="tiny")
                for k in range(8):
                    nc.tensor.matmul(
                        ps_g[:],
                        gw2_r[:, k * E : (k + 1) * E],
                        h1[:, k * S : (k + 1) * S],
                        start=(k == 0),
                        stop=(k == 7),
                    )
                gsum = pool.tile([E, 1], F32, tag="gsum")
                nc.vector.reduce_sum(gsum[:], ps_g[:], axis=mybir.AxisListType.X)
                gb2_sb = pool.tile([E, 1], F32, tag="gb2")
                nc.sync.dma_start(gb2_sb[:], gb2[:, None])
                gmean = pool.tile([E, 1], F32, tag="gmean")
                nc.vector.tensor_scalar_mul(gmean[:], gsum[:], 1.0 / S)
                gown = pool.tile([E, 1], F32, tag="gown")
                nc.vector.tensor_add(gown[:], gmean[:], gb2_sb[:])
                nc.sync.dma_start(ag_in[:], gown[:, 0])

            nc.gpsimd.collective_compute(
                "AllGather",
                mybir.AluOpType.bypass,
                ins=[ag_in[:]],
                outs=[ag_out[:]],
                replica_groups=groups,
            )
            if debug:
                nc.sync.dma_start(dbg_g[:], ag_out[:])

            # g broadcast across partitions [P, B*E]; transposed tiny [E, B]
            g_bc = pool.tile([P, NC * E], F32, tag="gbc")
            nc.sync.dma_start(
                g_bc[:], ag_out.ap()[None, :].broadcast_to([P, NC * E])
            )
            gT_r = pool.tile([E, B], F32R, tag="gT")
            nc.gpsimd.dma_start(gT_r[:], ag_out.rearrange("(b e) -> e b", e=E))

            # identity for the scaled-diag merge trick
            eye = pool.tile([P, P], F32, tag="eye")
            make_identity(nc, eye[:])

            # ---- merged per-sample biases ----
            # b1T[:, mt*8+b] = (g[b] @ eb1s)[mt-tile]      (full value)
            # b2T[:, m*8+b]  = (g[b] @ eb2)[m-tile] / 8    (1/8: summed by RS)
            eb1_r = pool.tile([E, DSL], F32R, tag="eb1")
            nc.sync.dma_start(eb1_r[:], eb1s[:])
            eb2_f = pool.tile([E, H], F32, tag="eb2f")
            nc.sync.dma_start(eb2_f[:], eb2[:])
            eb2_r8 = pool.tile([E, H], F32R, tag="eb2r")
            nc.scalar.activation(eb2_r8[:], eb2_f[:], AF.Copy, scale=1.0 / NC)
            b1t = pool.tile([P, 4 * B], F32, tag="b1t")
            b2t = pool.tile([P, 8 * B], F32, tag="b2t")
            for mt in range(4):
                ps = pp.tile([P, B], F32, tag="tiny")
                nc.tensor.matmul(
                    ps[:],
                    eb1_r[:, mt * P : (mt + 1) * P],
                    gT_r[:],
                    start=True,
                    stop=True,
                )
                nc.vector.tensor_copy(b1t[:, mt * B : (mt + 1) * B], ps[:])
            for m in range(8):
                ps = pp.tile([P, B], F32, tag="tiny")
                nc.tensor.matmul(
                    ps[:],
                    eb2_r8[:, m * P : (m + 1) * P],
                    gT_r[:],
                    start=True,
                    stop=True,
                )
                nc.vector.tensor_copy(b2t[:, m * B : (m + 1) * B], ps[:])

            def make_gdiag(b):
                tiles = []
                for e in range(E):
                    gd = pool.tile([P, P], F32R, tag="gd", bufs=16)
                    nc.scalar.activation(
                        gd[:],
                        eye[:],
                        AF.Copy,
                        scale=g_bc[:, b * E + e : b * E + e + 1],
                    )
                    tiles.append(gd)
                return tiles

            def phase1():
                HF = 256
                with tc.tile_pool(name="ew1p", bufs=1) as wp:
                  for p in range(2):
                    if True:
                        ew_tiles = []
                        for k in range(8):
                            ewk = wp.tile(
                                [P, E * HF], F32R, tag="ew1", bufs=10
                            )
                            nc.sync.dma_start(ewk[:], ew1d[p, k])
                            ew_tiles.append(ewk)

                        state = {}

                        def merge1(b):
                            gd = make_gdiag(b)
                            xb = pool.tile([P, 8 * S], F32R, tag="xb", bufs=2)
                            nc.sync.dma_start(
                                xb[:].rearrange("p (k s) -> p k s", k=8),
                                xt_all.rearrange("b (k p) s -> b p k s", p=P)[b],
                            )
                            w1t = pool.tile([P, 8 * HF], F32R, tag="wmt", bufs=2)
                            for k in range(8):
                                ps = pp.tile([P, HF], F32, tag="mm")
                                for e in range(E):
                                    nc.tensor.matmul(
                                        ps[:],
                                        gd[e][:],
                                        ew_tiles[k][
                                            :, e * HF : (e + 1) * HF
                                        ],
                                        start=(e == 0),
                                        stop=(e == E - 1),
                                    )
                                nc.vector.tensor_copy(
                                    w1t[:, k * HF : (k + 1) * HF], ps[:]
                                )
                            state[b] = (w1t, xb)

                        def gemm1(b):
                            w1t, xb = state.pop(b)
                            if debug and p == 0 and b == 0:
                                nc.sync.dma_start(dbg_w1t[:], w1t[:].bitcast(F32))
                            for m in range(2):
                                mt = p * 2 + m
                                ps = pp.tile([P, S], F32, tag="out")
                                for k in range(8):
                                    nc.tensor.matmul(
                                        ps[:],
                                        w1t[
                                            :, k * HF + m * P : k * HF + (m + 1) * P
                                        ],
                                        xb[:, k * S : (k + 1) * S],
                                        start=(k == 0),
                                        stop=(k == 7),
                                    )
                                y1 = pool.tile([P, S], F32R, tag="y1", bufs=4)
                                nc.scalar.activation(
                                    y1[:],
                                    ps[:],
                                    AF.Relu,
                                    bias=b1t[:, mt * B + b : mt * B + b + 1],
                                )
                                nc.sync.dma_start(y1_dram[b, mt], y1[:])

                        for b in range(B + 1):
                            if b < B:
                                merge1(b)
                            if b >= 1:
                                gemm1(b - 1)

            def phase2(with_rs=True):
                HH = 512
                with tc.tile_pool(name="ew2p", bufs=1) as wp:
                  for p in range(2):
                    rs_in = rs_in0 if p == 0 else rs_in1
                    if True:
                        ew_tiles = []
                        for kt in range(4):
                            ewk = wp.tile(
                                [P, E * HH], F32R, tag="ew2", bufs=5
                            )
                            nc.sync.dma_start(ewk[:], ew2d[p, kt])
                            ew_tiles.append(ewk)

                        state2 = {}

                        def merge2(b):
                            gd = make_gdiag(b)
                            yb = pool.tile([P, 4 * S], F32R, tag="yb", bufs=2)
                            nc.sync.dma_start(
                                yb[:].rearrange("p (k s) -> p k s", k=4),
                                y1_dram.rearrange("b k p s -> b p k s")[b],
                            )
                            w2t = pool.tile([P, 4 * HH], F32R, tag="wmt", bufs=2)
                            for kt in range(4):
                                ps = pp.tile([P, HH], F32, tag="mm")
                                for e in range(E):
                                    nc.tensor.matmul(
                                        ps[:],
                                        gd[e][:],
                                        ew_tiles[kt][
                                            :, e * HH : (e + 1) * HH
                                        ],
                                        start=(e == 0),
                                        stop=(e == E - 1),
                                    )
                                nc.vector.tensor_copy(
                                    w2t[:, kt * HH : (kt + 1) * HH], ps[:]
                                )
                            state2[b] = (w2t, yb)

                        def gemm2(b):
                            w2t, yb = state2.pop(b)
                            for m in range(4):
                                mg = p * 4 + m
                                ps = pp.tile([P, S], F32, tag="out")
                                for kt in range(4):
                                    nc.tensor.matmul(
                                        ps[:],
                                        w2t[
                                            :,
                                            kt * HH + m * P : kt * HH + (m + 1) * P,
                                        ],
                                        yb[:, kt * S : (kt + 1) * S],
                                        start=(kt == 0),
                                        stop=(kt == 3),
                                    )
                                y2 = pool.tile([P, S], F32, tag="y2", bufs=4)
                                nc.scalar.activation(
                                    y2[:],
                                    ps[:],
                                    AF.Identity,
                                    bias=b2t[:, mg * B + b : mg * B + b + 1],
                                )
                                nc.sync.dma_start(rs_in[b, m], y2[:])

                        for b in range(B + 1):
                            if b < B:
                                merge2(b)
                            if b >= 1:
                                gemm2(b - 1)

                    if with_rs:
                        nc.gpsimd.collective_compute(
                            "ReduceScatter",
                            mybir.AluOpType.add,
                            ins=[rs_in.ap().rearrange("b m p s -> (b m p) s")],
                            outs=[(rs_out0 if p == 0 else rs_out1)[:]],
                            replica_groups=groups,
                        )

            if time_loop:
                with tc.For_i(0, time_loop, 1):
                    phase1()
                    phase2(with_rs=False)
                nc.sync.dma_start(y_out[0 : 4 * P], rs_in0.ap()[0])
                nc.sync.dma_start(y_out[4 * P : 8 * P], rs_in1.ap()[0])
            else:
                phase1()
                phase2(with_rs=True)
                if debug:
                    for mt in range(4):
                        nc.sync.dma_start(dbg_y1[mt], y1_dram[0, mt].bitcast(F32))
                    for m in range(4):
                        nc.sync.dma_start(dbg_rs0[m], rs_in0[0, m])
                nc.sync.dma_start(y_out[0 : 4 * P], rs_out0[:])
                nc.sync.dma_start(y_out[4 * P : 8 * P], rs_out1[:])

    nc.compile()
    return nc


def _ew1_dev(a):
    # a: [E, DSL(o), H(i)] -> [2pass, 8k, 128p(i), 8e * 256o]
    a2 = np.ascontiguousarray(np.asarray(a, np.float32).transpose(2, 0, 1))
    a3 = a2.reshape(8, P, E, 2, 256).transpose(3, 0, 1, 2, 4)
    return np.ascontiguousarray(a3.reshape(2, 8, P, E * 256))


def _ew2_dev(c):
    # c: [E, H(h), DSL(d)] -> [2pass, 4kt, 128p(d), 8e * 512h]
    c2 = np.ascontiguousarray(np.asarray(c, np.float32).transpose(2, 0, 1))
    c3 = c2.reshape(4, P, E, 2, 512).transpose(3, 0, 1, 2, 4)
    return np.ascontiguousarray(c3.reshape(2, 4, P, E * 512))


def _shard_inputs(hidden_states, gW1, gb1, gW2, gb2, eW1, eb1, eW2, eb2):
    xt_all = np.ascontiguousarray(
        np.asarray(hidden_states, dtype=np.float32).transpose(1, 2, 0)
    )  # [B, H, S]
    gW1t = np.ascontiguousarray(np.asarray(gW1, np.float32).T)
    gb1t = np.ascontiguousarray(np.asarray(gb1, np.float32).reshape(8, P).T)
    gW2t = np.ascontiguousarray(np.asarray(gW2, np.float32).T)
    gb2 = np.ascontiguousarray(np.asarray(gb2, np.float32))
    eW1 = np.asarray(eW1, np.float32)
    eW2 = np.asarray(eW2, np.float32)
    eb1 = np.asarray(eb1, np.float32)
    eb2 = np.ascontiguousarray(np.asarray(eb2, np.float32))
    in_maps = []
    for j in range(NC):
        sl = slice(j * DSL, (j + 1) * DSL)
        in_maps.append(
            {
                "xt_all": xt_all,
                "xt_own": np.ascontiguousarray(xt_all[j]),
                "gw1t": gW1t,
                "gb1t": gb1t,
                "gw2t": gW2t,
                "gb2": gb2,
                "ew1d": _ew1_dev(eW1[:, sl, :]),
                "ew2d": _ew2_dev(eW2[:, :, sl]),
                "eb1s": np.ascontiguousarray(eb1[:, sl]),
                "eb2": eb2,
            }
        )
    return in_maps


# ---------------- SPMD runner (persistent jit over axon PJRT) -----------

_CACHE = {}


def _build_runner(debug=False, time_loop=0):
    import jax
    from jax.sharding import Mesh, PartitionSpec
    from jax.experimental.shard_map import shard_map
    from concourse import bass2jax

    nc = build_module(debug=debug, time_loop=time_loop)
    bass2jax.install_neuronx_cc_hook()
    partition_name = nc.partition_id_tensor.name if nc.partition_id_tensor else None

    in_names, out_names, out_avals = [], [], []
    for alloc in nc.m.functions[0].allocations:
        if not isinstance(alloc, mybir.MemoryLocationSet):
            continue
        name = alloc.memorylocations[0].name
        if alloc.kind == "ExternalInput":
            if name != partition_name:
                in_names.append(name)
        elif alloc.kind == "ExternalOutput":
            out_avals.append(
                jax.core.ShapedArray(
                    tuple(alloc.tensor_shape), mybir.dt.np(alloc.dtype)
                )
            )
            out_names.append(name)
    n_outs = len(out_names)
    all_in_names = list(in_names) + list(out_names)
    if partition_name is not None:
        all_in_names.append(partition_name)

    def _body(*args):
        operands = list(args)
        if partition_name is not None:
            operands.append(bass2jax.partition_id_tensor())
        return tuple(
            bass2jax._bass_exec_p.bind(
                *operands,
                out_avals=tuple(out_avals),
                in_names=tuple(all_in_names),
                out_names=tuple(out_names),
                lowering_input_output_aliases=(),
                sim_require_finite=True,
                sim_require_nnan=True,
                nc=nc,
            )
        )

    devices = jax.devices()[:NC]
    mesh = Mesh(np.asarray(devices), ("core",))
    n_params = len(in_names)
    sharded = jax.jit(
        shard_map(
            _body,
            mesh=mesh,
            in_specs=(PartitionSpec("core"),) * (n_params + n_outs),
            out_specs=(PartitionSpec("core"),) * n_outs,
            check_rep=False,
        ),
        keep_unused=True,
    )
    zero_shapes = [((NC * a.shape[0], *a.shape[1:]), a.dtype) for a in out_avals]

    def run(in_maps, device_inputs=None, fetch=True):
        if device_inputs is None:
            concat_in = [
                np.concatenate(
                    [np.asarray(in_maps[c][n]) for c in range(NC)], axis=0
                )
                for n in in_names
            ]
            dev_params = [jax.device_put(x) for x in concat_in]
            dev_zeros = [jax.device_put(np.zeros(s, d)) for s, d in zero_shapes]
            device_inputs = (dev_params, dev_zeros)
            jax.block_until_ready(dev_params)
            jax.block_until_ready(dev_zeros)
        dev_params, dev_zeros = device_inputs
        out_arrs = sharded(*dev_params, *dev_zeros)
        jax.block_until_ready(out_arrs)
        if not fetch:
            return None, device_inputs
        results = [
            {
                name: np.asarray(out_arrs[i]).reshape(NC, *out_avals[i].shape)[c]
                for i, name in enumerate(out_names)
            }
            for c in range(NC)
        ]
        return results, device_inputs

    return run


def get_runner(debug=False, time_loop=0):
    key = ("run", debug, time_loop)
    if key not in _CACHE:
        _CACHE[key] = _build_runner(debug=debug, time_loop=time_loop)
    return _CACHE[key]


def kernel(**inputs) -> np.ndarray:
    run = get_runner()
    in_maps = _shard_inputs(**inputs)
    results, _ = run(in_maps)
    # core b's output is y2^T[b] = [H, S]; assemble [S, B, H]
    y2t = np.stack([results[b]["y2t"] for b in range(B)], axis=0)  # [B, H, S]
    return np.ascontiguousarray(y2t.transpose(2, 0, 1)).astype(np.float32)


# revision 18
# speedup vs baseline: 1055.7402x; 844.1659x over previous
"""Trainium2 Bass kernel for nn_DictMoEDirect (moe_routing).

Reference computation (fp32, shapes hardcoded):
  x = hidden_states.transpose(1,0,2)              # [B,S,H]
  g = mean_s(relu(x@gW1.T + gb1) @ gW2.T + gb2)   # [B,E]
  W1_b = sum_e g[b,e] eW1[e]; b1_b = g[b]@eb1     # per-sample merged MLP
  W2_b = sum_e g[b,e] eW2[e]; b2_b = g[b]@eb2
  y = relu(x@W1_b.T + b1_b) @ W2_b.T + b2_b       # [B,S,H]
  return y.transpose(1,0,2)                       # [S,B,H]

Distribution over 8 NeuronCores:
  - Gate: data-parallel (core b computes g[b] from its own sample), then a
    tiny AllGather of g (256 B).
  - Expert FFN: tensor-parallel over DFF. Core j owns DFF slice j (512 wide).
    It merges its slice of W1/W2 for ALL samples with the identity-scaled
    matmul trick on the PE (lhsT = diag(g[b,e]), accumulated over e in PSUM),
    computes y1[:, dff_j] for all samples, then layer-2 partial products,
    summed across cores with two ReduceScatters (one per H-half so the first
    overlaps the second half's compute).
  All matmuls run in float32r (TF32) at 1 cycle/row with fp32 PSUM accum.

kernel(**inputs) takes full unsharded inputs, shards/transposes on the host,
runs the SPMD kernel, and reassembles the full [S,B,H] output.
"""

import numpy as np

import concourse.bass as bass  # noqa: F401
import concourse.mybir as mybir
from concourse import bacc
from concourse.tile import TileContext
from concourse.masks import make_identity

H = 1024
DFF = 4096
E = 8
B = 8
S = 512
NC = 8
DSL = DFF // NC  # 512, per-core DFF slice
P = 128
F32 = mybir.dt.float32
F32R = mybir.dt.float32r
AF = mybir.ActivationFunctionType


def build_module(debug=False, time_loop=0):
    """time_loop=R wraps the FFN phases (not gate/collectives) in an
    on-device For loop for timing runs; outputs are then meaningless."""
    nc = bacc.Bacc()

    # ---- I/O ----
    xt_all = nc.declare_dram_parameter("xt_all", [B, H, S], F32R, isOutput=False)
    xt_own = nc.declare_dram_parameter("xt_own", [H, S], F32R, isOutput=False)
    gw1t = nc.declare_dram_parameter("gw1t", [H, H], F32R, isOutput=False)
    gb1t = nc.declare_dram_parameter("gb1t", [P, 8], F32, isOutput=False)
    gw2t = nc.declare_dram_parameter("gw2t", [H, E], F32R, isOutput=False)
    gb2 = nc.declare_dram_parameter("gb2", [E], F32, isOutput=False)
    ew1d = nc.declare_dram_parameter("ew1d", [2, 8, P, E * 256], F32R, isOutput=False)
    ew2d = nc.declare_dram_parameter("ew2d", [2, 4, P, E * 512], F32R, isOutput=False)
    eb1s = nc.declare_dram_parameter("eb1s", [E, DSL], F32R, isOutput=False)
    eb2 = nc.declare_dram_parameter("eb2", [E, H], F32, isOutput=False)
    y_out = nc.declare_dram_parameter("y2t", [H, S], F32, isOutput=True)
    if debug:
        dbg_g = nc.declare_dram_parameter("dbg_g", [NC * E], F32, isOutput=True)
        dbg_y1 = nc.declare_dram_parameter("dbg_y1", [4, P, S], F32, isOutput=True)
        dbg_w1t = nc.declare_dram_parameter(
            "dbg_w1t", [P, 8 * 256], F32, isOutput=True
        )
        dbg_rs0 = nc.declare_dram_parameter("dbg_rs0", [4, P, S], F32, isOutput=True)

    # ---- internal DRAM ----
    ag_in = nc.dram_tensor("ag_in", [E], F32)
    ag_out = nc.dram_tensor("ag_out", [NC * E], F32, addr_space="Shared")
    y1_dram = nc.dram_tensor("y1_dram", [B, 4, P, S], F32R)
    rs_in0 = nc.dram_tensor("rs_in0", [B, 4, P, S], F32)
    rs_in1 = nc.dram_tensor("rs_in1", [B, 4, P, S], F32)
    rs_out0 = nc.dram_tensor("rs_out0", [4 * P, S], F32)
    rs_out1 = nc.dram_tensor("rs_out1", [4 * P, S], F32)
    groups = [list(range(NC))]

    with TileContext(nc) as tc:
        with (
            tc.tile_pool(name="main", bufs=1) as pool,
            tc.tile_pool(name="psum", bufs=2, space="PSUM") as pp,
        ):
            # ---------------- gate (own sample) ----------------
            xo = pool.tile([P, 8 * S], F32R, tag="xb", bufs=2)
            nc.sync.dma_start(
                xo[:].rearrange("p (k s) -> p k s", k=8),
                xt_own.rearrange("(k p) s -> p k s", p=P),
            )
            gb1_sb = pool.tile([P, 8], F32, tag="gb1")
            nc.sync.dma_start(gb1_sb[:], gb1t[:])
            h1 = pool.tile([P, 8 * S], F32R, tag="xb", bufs=2)
            with tc.tile_pool(name="gatew", bufs=1) as gwpool:
                gw1_r = pool.tile([P, 8 * H], F32R, tag="ew", bufs=3)
                for k in range(8):
                    nc.sync.dma_start(
                        gw1_r[:, k * H : (k + 1) * H],
                        gw1t[k * P : (k + 1) * P, :],
                    )
                for m in range(8):
                    ps = pp.tile([P, S], F32, tag="out")
                    for k in range(8):
                        nc.tensor.matmul(
                            ps[:],
                            gw1_r[:, k * H + m * P : k * H + (m + 1) * P],
                            xo[:, k * S : (k + 1) * S],
                            start=(k == 0),
                            stop=(k == 7),
                        )
                    nc.scalar.activation(
                        h1[:, m * S : (m + 1) * S],
                        ps[:],
                        AF.Relu,
                        bias=gb1_sb[:, m : m + 1],
                    )
                gw2_r = pool.tile([P, 64], F32R, tag="gw2")
                for k in range(8):
                    nc.sync.dma_start(
                        gw2_r[:, k * E : (k + 1) * E],
                        gw2t[k * P : (k + 1) * P, :],
                    )
                ps_g = pp.tile([E, S], F32, tag="tiny")
                for k in range(8):
                    nc.tensor.matmul(
                        ps_g[:],
                        gw2_r[:, k * E : (k + 1) * E],
                        h1[:, k * S : (k + 1) * S],
                        start=(k == 0),
                        stop=(k == 7),
                    )
                gsum = pool.tile([E, 1], F32, tag="gsum")
                nc.vector.reduce_sum(gsum[:], ps_g[:], axis=mybir.AxisListType.X)
                gb2_sb = pool.tile([E, 1], F32, tag="gb2")
                nc.sync.dma_start(gb2_sb[:], gb2[:, None])
                gmean = pool.tile([E, 1], F32, tag="gmean")
                nc.vector.tensor_scalar_mul(gmean[:], gsum[:], 1.0 / S)
                gown = pool.tile([E, 1], F32, tag="gown")
                nc.vector.tensor_add(gown[:], gmean[:], gb2_sb[:])
                nc.sync.dma_start(ag_in[:], gown[:, 0])

            nc.gpsimd.collective_compute(
                "AllGather",
                mybir.AluOpType.bypass,
                ins=[ag_in[:]],
                outs=[ag_out[:]],
                replica_groups=groups,
            )
            if debug:
                nc.sync.dma_start(dbg_g[:], ag_out[:])

            # g broadcast across partitions [P, B*E]; transposed tiny [E, B]
            g_bc = pool.tile([P, NC * E], F32, tag="gbc")
            nc.sync.dma_start(
                g_bc[:], ag_out.ap()[None, :].broadcast_to([P, NC * E])
            )
            gT_r = pool.tile([E, B], F32R, tag="gT")
            nc.gpsimd.dma_start(gT_r[:], ag_out.rearrange("(b e) -> e b", e=E))

            # identity for the scaled-diag merge trick
            eye = pool.tile([P, P], F32, tag="eye")
            make_identity(nc, eye[:])

            # ---- merged per-sample biases ----
            # b1T[:, mt*8+b] = (g[b] @ eb1s)[mt-tile]      (full value)
            # b2T[:, m*8+b]  = (g[b] @ eb2)[m-tile] / 8    (1/8: summed by RS)
            eb1_r = pool.tile([E, DSL], F32R, tag="eb1")
            nc.sync.dma_start(eb1_r[:], eb1s[:])
            eb2_f = pool.tile([E, H], F32, tag="eb2f")
            nc.sync.dma_start(eb2_f[:], eb2[:])
            eb2_r8 = pool.tile([E, H], F32R, tag="eb2r")
            nc.scalar.activation(eb2_r8[:], eb2_f[:], AF.Copy, scale=1.0 / NC)
            b1t = pool.tile([P, 4 * B], F32, tag="b1t")
            b2t = pool.tile([P, 8 * B], F32, tag="b2t")
            for mt in range(4):
                ps = pp.tile([P, B], F32, tag="tiny")
                nc.tensor.matmul(
                    ps[:],
                    eb1_r[:, mt * P : (mt + 1) * P],
                    gT_r[:],
                    start=True,
                    stop=True,
                )
                nc.vector.tensor_copy(b1t[:, mt * B : (mt + 1) * B], ps[:])
            for m in range(8):
                ps = pp.tile([P, B], F32, tag="tiny")
                nc.tensor.matmul(
                    ps[:],
                    eb2_r8[:, m * P : (m + 1) * P],
                    gT_r[:],
                    start=True,
                    stop=True,
                )
                nc.vector.tensor_copy(b2t[:, m * B : (m + 1) * B], ps[:])

            def make_gdiag(b):
                tiles = []
                for e in range(E):
                    gd = pool.tile([P, P], F32R, tag="gd", bufs=8)
                    nc.scalar.activation(
                        gd[:],
                        eye[:],
                        AF.Copy,
                        scale=g_bc[:, b * E + e : b * E + e + 1],
                    )
                    tiles.append(gd)
                return tiles

            def phase1():
                HF = 256
                for p in range(2):
                    if True:
                        ewA = pool.tile([P, 4 * E * HF], F32R, tag="ew", bufs=3)
                        ewB = pool.tile([P, 4 * E * HF], F32R, tag="ew", bufs=3)
                        for k in range(4):
                            nc.sync.dma_start(
                                ewA[:, k * E * HF : (k + 1) * E * HF],
                                ew1d[p, k],
                            )
                        for k in range(4):
                            nc.sync.dma_start(
                                ewB[:, k * E * HF : (k + 1) * E * HF],
                                ew1d[p, 4 + k],
                            )

                        state = {}

                        def merge1(b):
                            gd = make_gdiag(b)
                            xb = pool.tile([P, 8 * S], F32R, tag="xb", bufs=2)
                            nc.sync.dma_start(
                                xb[:].rearrange("p (k s) -> p k s", k=8),
                                xt_all.rearrange("b (k p) s -> b p k s", p=P)[b],
                            )
                            w1t = pool.tile([P, 8 * HF], F32R, tag="wmt", bufs=4)
                            for k in range(8):
                                ps = pp.tile([P, HF], F32, tag="mm")
                                for e in range(E):
                                    nc.tensor.matmul(
                                        ps[:],
                                        gd[e][:],
                                        (ewA if k < 4 else ewB)[
                                            :,
                                            ((k % 4) * E + e)
                                            * HF : ((k % 4) * E + e + 1)
                                            * HF,
                                        ],
                                        start=(e == 0),
                                        stop=(e == E - 1),
                                    )
                                nc.vector.tensor_copy(
                                    w1t[:, k * HF : (k + 1) * HF], ps[:]
                                )
                            state[b] = (w1t, xb)

                        def gemm1(b):
                            w1t, xb = state.pop(b)
                            if debug and p == 0 and b == 0:
                                nc.sync.dma_start(dbg_w1t[:], w1t[:].bitcast(F32))
                            for m in range(2):
                                mt = p * 2 + m
                                ps = pp.tile([P, S], F32, tag="out")
                                for k in range(8):
                                    nc.tensor.matmul(
                                        ps[:],
                                        w1t[
                                            :, k * HF + m * P : k * HF + (m + 1) * P
                                        ],
                                        xb[:, k * S : (k + 1) * S],
                                        start=(k == 0),
                                        stop=(k == 7),
                                    )
                                y1 = pool.tile([P, S], F32R, tag="y1", bufs=4)
                                nc.scalar.activation(
                                    y1[:],
                                    ps[:],
                                    AF.Relu,
                                    bias=b1t[:, mt * B + b : mt * B + b + 1],
                                )
                                nc.sync.dma_start(y1_dram[b, mt], y1[:])

                        for b in range(B + 1):
                            if b < B:
                                merge1(b)
                            if b >= 1:
                                gemm1(b - 1)

            def phase2(with_rs=True):
                HH = 512
                for p in range(2):
                    rs_in = rs_in0 if p == 0 else rs_in1
                    if True:
                        ewA = pool.tile([P, 2 * E * HH], F32R, tag="ew", bufs=3)
                        ewB = pool.tile([P, 2 * E * HH], F32R, tag="ew", bufs=3)
                        for kt in range(2):
                            nc.sync.dma_start(
                                ewA[:, kt * E * HH : (kt + 1) * E * HH],
                                ew2d[p, kt],
                            )
                        for kt in range(2):
                            nc.sync.dma_start(
                                ewB[:, kt * E * HH : (kt + 1) * E * HH],
                                ew2d[p, 2 + kt],
                            )

                        state2 = {}

                        def merge2(b):
                            gd = make_gdiag(b)
                            yb = pool.tile([P, 4 * S], F32R, tag="wmt", bufs=4)
                            nc.sync.dma_start(
                                yb[:].rearrange("p (k s) -> p k s", k=4),
                                y1_dram.rearrange("b k p s -> b p k s")[b],
                            )
                            w2t = pool.tile([P, 4 * HH], F32R, tag="wmt", bufs=4)
                            for kt in range(4):
                                ps = pp.tile([P, HH], F32, tag="mm")
                                for e in range(E):
                                    nc.tensor.matmul(
                                        ps[:],
                                        gd[e][:],
                                        (ewA if kt < 2 else ewB)[
                                            :,
                                            ((kt % 2) * E + e)
                                            * HH : ((kt % 2) * E + e + 1)
                                            * HH,
                                        ],
                                        start=(e == 0),
                                        stop=(e == E - 1),
                                    )
                                nc.vector.tensor_copy(
                                    w2t[:, kt * HH : (kt + 1) * HH], ps[:]
                                )
                            state2[b] = (w2t, yb)

                        def gemm2(b):
                            w2t, yb = state2.pop(b)
                            for m in range(4):
                                mg = p * 4 + m
                                ps = pp.tile([P, S], F32, tag="out")
                                for kt in range(4):
                                    nc.tensor.matmul(
                                        ps[:],
                                        w2t[
                                            :,
                                            kt * HH + m * P : kt * HH + (m + 1) * P,
                                        ],
                                        yb[:, kt * S : (kt + 1) * S],
                                        start=(kt == 0),
                                        stop=(kt == 3),
                                    )
                                y2 = pool.tile([P, S], F32, tag="y2", bufs=4)
                                nc.scalar.activation(
                                    y2[:],
                                    ps[:],
                                    AF.Identity,
                                    bias=b2t[:, mg * B + b : mg * B + b + 1],
                                )
                                nc.sync.dma_start(rs_in[b, m], y2[:])

                        for b in range(B + 1):
                            if b < B:
                                merge2(b)
                            if b >= 1:
                                gemm2(b - 1)

                    if with_rs:
                        nc.gpsimd.collective_compute(
                            "ReduceScatter",
                            mybir.AluOpType.add,
                            ins=[rs_in.ap().rearrange("b m p s -> (b m p) s")],
                            outs=[(rs_out0 if p == 0 else rs_out1)[:]],
                            replica_groups=groups,
                        )

            if time_loop:
                with tc.For_i(0, time_loop, 1):
                    phase1()
                    phase2(with_rs=False)
                nc.sync.dma_start(y_out[0 : 4 * P], rs_in0.ap()[0])
                nc.sync.dma_start(y_out[4 * P : 8 * P], rs_in1.ap()[0])
            else:
                phase1()
                phase2(with_rs=True)
                if debug:
                    for mt in range(4):
                        nc.sync.dma_start(dbg_y1[mt], y1_dram[0, mt].bitcast(F32))
                    for m in range(4):
                        nc.sync.dma_start(dbg_rs0[m], rs_in0[0, m])
                nc.sync.dma_start(y_out[0 : 4 * P], rs_out0[:])
                nc.sync.dma_start(y_out[4 * P : 8 * P], rs_out1[:])

    nc.compile()
    return nc


def _ew1_dev(a):
    # a: [E, DSL(o), H(i)] -> [2pass, 8k, 128p(i), 8e * 256o]
    a2 = np.ascontiguousarray(np.asarray(a, np.float32).transpose(2, 0, 1))
    a3 = a2.reshape(8, P, E, 2, 256).transpose(3, 0, 1, 2, 4)
    return np.ascontiguousarray(a3.reshape(2, 8, P, E * 256))


def _ew2_dev(c):
    # c: [E, H(h), DSL(d)] -> [2pass, 4kt, 128p(d), 8e * 512h]
    c2 = np.ascontiguousarray(np.asarray(c, np.float32).transpose(2, 0, 1))
    c3 = c2.reshape(4, P, E, 2, 512).transpose(3, 0, 1, 2, 4)
    return np.ascontiguousarray(c3.reshape(2, 4, P, E * 512))


def _shard_inputs(hidden_states, gW1, gb1, gW2, gb2, eW1, eb1, eW2, eb2):
    xt_all = np.ascontiguousarray(
        np.asarray(hidden_states, dtype=np.float32).transpose(1, 2, 0)
    )  # [B, H, S]
    gW1t = np.ascontiguousarray(np.asarray(gW1, np.float32).T)
    gb1t = np.ascontiguousarray(np.asarray(gb1, np.float32).reshape(8, P).T)
    gW2t = np.ascontiguousarray(np.asarray(gW2, np.float32).T)
    gb2 = np.ascontiguousarray(np.asarray(gb2, np.float32))
    eW1 = np.asarray(eW1, np.float32)
    eW2 = np.asarray(eW2, np.float32)
    eb1 = np.asarray(eb1, np.float32)
    eb2 = np.ascontiguousarray(np.asarray(eb2, np.float32))
    in_maps = []
    for j in range(NC):
        sl = slice(j * DSL, (j + 1) * DSL)
        in_maps.append(
            {
                "xt_all": xt_all,
                "xt_own": np.ascontiguousarray(xt_all[j]),
                "gw1t": gW1t,
                "gb1t": gb1t,
                "gw2t": gW2t,
                "gb2": gb2,
                "ew1d": _ew1_dev(eW1[:, sl, :]),
                "ew2d": _ew2_dev(eW2[:, :, sl]),
                "eb1s": np.ascontiguousarray(eb1[:, sl]),
                "eb2": eb2,
            }
        )
    return in_maps


# ---------------- SPMD runner (persistent jit over axon PJRT) -----------

_CACHE = {}


def _build_runner(debug=False, time_loop=0):
    import jax
    from jax.sharding import Mesh, PartitionSpec
    from jax.experimental.shard_map import shard_map
    from concourse import bass2jax

    nc = build_module(debug=debug, time_loop=time_loop)
    bass2jax.install_neuronx_cc_hook()
    partition_name = nc.partition_id_tensor.name if nc.partition_id_tensor else None

    in_names, out_names, out_avals = [], [], []
    for alloc in nc.m.functions[0].allocations:
        if not isinstance(alloc, mybir.MemoryLocationSet):
            continue
        name = alloc.memorylocations[0].name
        if alloc.kind == "ExternalInput":
            if name != partition_name:
                in_names.append(name)
        elif alloc.kind == "ExternalOutput":
            out_avals.append(
                jax.core.ShapedArray(
                    tuple(alloc.tensor_shape), mybir.dt.np(alloc.dtype)
                )
            )
            out_names.append(name)
    n_outs = len(out_names)
    all_in_names = list(in_names) + list(out_names)
    if partition_name is not None:
        all_in_names.append(partition_name)

    def _body(*args):
        operands = list(args)
        if partition_name is not None:
            operands.append(bass2jax.partition_id_tensor())
        return tuple(
            bass2jax._bass_exec_p.bind(
                *operands,
                out_avals=tuple(out_avals),
                in_names=tuple(all_in_names),
                out_names=tuple(out_names),
                lowering_input_output_aliases=(),
                sim_require_finite=True,
                sim_require_nnan=True,
                nc=nc,
            )
        )

    devices = jax.devices()[:NC]
    mesh = Mesh(np.asarray(devices), ("core",))
    n_params = len(in_names)
    sharded = jax.jit(
        shard_map(
            _body,
            mesh=mesh,
            in_specs=(PartitionSpec("core"),) * (n_params + n_outs),
            out_specs=(PartitionSpec("core"),) * n_outs,
            check_rep=False,
        ),
        keep_unused=True,
    )
    zero_shapes = [((NC * a.shape[0], *a.shape[1:]), a.dtype) for a in out_avals]

    def run(in_maps, device_inputs=None, fetch=True):
        if device_inputs is None:
            concat_in = [
                np.concatenate(
                    [np.asarray(in_maps[c][n]) for c in range(NC)], axis=0
                )
                for n in in_names
            ]
            dev_params = [jax.device_put(x) for x in concat_in]
            dev_zeros = [jax.device_put(np.zeros(s, d)) for s, d in zero_shapes]
            device_inputs = (dev_params, dev_zeros)
            jax.block_until_ready(dev_params)
            jax.block_until_ready(dev_zeros)
        dev_params, dev_zeros = device_inputs
        out_arrs = sharded(*dev_params, *dev_zeros)
        jax.block_until_ready(out_arrs)
        if not fetch:
            return None, device_inputs
        results = [
            {
                name: np.asarray(out_arrs[i]).reshape(NC, *out_avals[i].shape)[c]
                for i, name in enumerate(out_names)
            }
            for c in range(NC)
        ]
        return results, device_inputs

    return run


def get_runner(debug=False, time_loop=0):
    key = ("run", debug, time_loop)
    if key not in _CACHE:
        _CACHE[key] = _build_runner(debug=debug, time_loop=time_loop)
    return _CACHE[key]


def kernel(**inputs) -> np.ndarray:
    run = get_runner()
    in_maps = _shard_inputs(**inputs)
    results, _ = run(in_maps)
    # core b's output is y2^T[b] = [H, S]; assemble [S, B, H]
    y2t = np.stack([results[b]["y2t"] for b in range(B)], axis=0)  # [B, H, S]
    return np.ascontiguousarray(y2t.transpose(2, 0, 1)).astype(np.float32)


# revision 19
# speedup vs baseline: 1094.4984x; 1.0367x over previous
"""Trainium2 Bass kernel for nn_DictMoEDirect (moe_routing).

Reference computation (fp32, shapes hardcoded):
  x = hidden_states.transpose(1,0,2)              # [B,S,H]
  g = mean_s(relu(x@gW1.T + gb1) @ gW2.T + gb2)   # [B,E]
  W1_b = sum_e g[b,e] eW1[e]; b1_b = g[b]@eb1     # per-sample merged MLP
  W2_b = sum_e g[b,e] eW2[e]; b2_b = g[b]@eb2
  y = relu(x@W1_b.T + b1_b) @ W2_b.T + b2_b       # [B,S,H]
  return y.transpose(1,0,2)                       # [S,B,H]

Distribution over 8 NeuronCores:
  - Gate: data-parallel (core b computes g[b] from its own sample), then a
    tiny AllGather of g (256 B).
  - Expert FFN: tensor-parallel over DFF. Core j owns DFF slice j (512 wide).
    It merges its slice of W1/W2 for ALL samples with the identity-scaled
    matmul trick on the PE (lhsT = diag(g[b,e]), accumulated over e in PSUM),
    computes y1[:, dff_j] for all samples, then layer-2 partial products,
    summed across cores with two ReduceScatters (one per H-half so the first
    overlaps the second half's compute).
  All matmuls run in float32r (TF32) at 1 cycle/row with fp32 PSUM accum.

kernel(**inputs) takes full unsharded inputs, shards/transposes on the host,
runs the SPMD kernel, and reassembles the full [S,B,H] output.
"""

import numpy as np

import concourse.bass as bass  # noqa: F401
import concourse.mybir as mybir
from concourse import bacc
from concourse.tile import TileContext
from concourse.masks import make_identity

H = 1024
DFF = 4096
E = 8
B = 8
S = 512
NC = 8
DSL = DFF // NC  # 512, per-core DFF slice
P = 128
F32 = mybir.dt.float32
F32R = mybir.dt.float32r
AF = mybir.ActivationFunctionType


def build_module(debug=False, time_loop=0):
    """time_loop=R wraps the FFN phases (not gate/collectives) in an
    on-device For loop for timing runs; outputs are then meaningless."""
    nc = bacc.Bacc()

    # ---- I/O ----
    xt_all = nc.declare_dram_parameter("xt_all", [B, H, S], F32R, isOutput=False)
    xt_own = nc.declare_dram_parameter("xt_own", [H, S], F32R, isOutput=False)
    gw1t = nc.declare_dram_parameter("gw1t", [H, H], F32R, isOutput=False)
    gb1t = nc.declare_dram_parameter("gb1t", [P, 8], F32, isOutput=False)
    gw2t = nc.declare_dram_parameter("gw2t", [H, E], F32R, isOutput=False)
    gb2 = nc.declare_dram_parameter("gb2", [E], F32, isOutput=False)
    ew1d = nc.declare_dram_parameter("ew1d", [2, 8, P, E * 256], F32R, isOutput=False)
    ew2d = nc.declare_dram_parameter("ew2d", [2, 4, P, E * 512], F32R, isOutput=False)
    eb1s = nc.declare_dram_parameter("eb1s", [E, DSL], F32R, isOutput=False)
    eb2 = nc.declare_dram_parameter("eb2", [E, H], F32, isOutput=False)
    y_out = nc.declare_dram_parameter("y2t", [H, S], F32, isOutput=True)
    if debug:
        dbg_g = nc.declare_dram_parameter("dbg_g", [NC * E], F32, isOutput=True)
        dbg_y1 = nc.declare_dram_parameter("dbg_y1", [4, P, S], F32, isOutput=True)
        dbg_w1t = nc.declare_dram_parameter(
            "dbg_w1t", [P, 8 * 256], F32, isOutput=True
        )
        dbg_rs0 = nc.declare_dram_parameter("dbg_rs0", [4, P, S], F32, isOutput=True)

    # ---- internal DRAM ----
    ag_in = nc.dram_tensor("ag_in", [E], F32)
    ag_out = nc.dram_tensor("ag_out", [NC * E], F32, addr_space="Shared")
    y1_dram = nc.dram_tensor("y1_dram", [B, 4, P, S], F32R)
    rs_in0 = nc.dram_tensor("rs_in0", [B, 4, P, S], F32)
    rs_in1 = nc.dram_tensor("rs_in1", [B, 4, P, S], F32)
    rs_out0 = nc.dram_tensor("rs_out0", [4 * P, S], F32)
    rs_out1 = nc.dram_tensor("rs_out1", [4 * P, S], F32)
    groups = [list(range(NC))]

    with TileContext(nc) as tc:
        with (
            tc.tile_pool(name="main", bufs=1) as pool,
            tc.tile_pool(name="psum", bufs=2, space="PSUM") as pp,
        ):
            # ---------------- gate (own sample) ----------------
            xo = pool.tile([P, 8 * S], F32R, tag="xb", bufs=2)
            nc.sync.dma_start(
                xo[:].rearrange("p (k s) -> p k s", k=8),
                xt_own.rearrange("(k p) s -> p k s", p=P),
            )
            gb1_sb = pool.tile([P, 8], F32, tag="gb1")
            nc.sync.dma_start(gb1_sb[:], gb1t[:])
            h1 = pool.tile([P, 8 * S], F32R, tag="xb", bufs=2)
            with tc.tile_pool(name="gatew", bufs=1) as gwpool:
                gw1_r = pool.tile([P, 8 * H], F32R, tag="ew", bufs=3)
                for k in range(8):
                    nc.sync.dma_start(
                        gw1_r[:, k * H : (k + 1) * H],
                        gw1t[k * P : (k + 1) * P, :],
                    )
                for m in range(8):
                    ps = pp.tile([P, S], F32, tag="out", bufs=3)
                    for k in range(8):
                        nc.tensor.matmul(
                            ps[:],
                            gw1_r[:, k * H + m * P : k * H + (m + 1) * P],
                            xo[:, k * S : (k + 1) * S],
                            start=(k == 0),
                            stop=(k == 7),
                        )
                    nc.scalar.activation(
                        h1[:, m * S : (m + 1) * S],
                        ps[:],
                        AF.Relu,
                        bias=gb1_sb[:, m : m + 1],
                    )
                gw2_r = pool.tile([P, 64], F32R, tag="gw2")
                for k in range(8):
                    nc.sync.dma_start(
                        gw2_r[:, k * E : (k + 1) * E],
                        gw2t[k * P : (k + 1) * P, :],
                    )
                ps_g = pp.tile([E, S], F32, tag="tiny")
                for k in range(8):
                    nc.tensor.matmul(
                        ps_g[:],
                        gw2_r[:, k * E : (k + 1) * E],
                        h1[:, k * S : (k + 1) * S],
                        start=(k == 0),
                        stop=(k == 7),
                    )
                gsum = pool.tile([E, 1], F32, tag="gsum")
                nc.vector.reduce_sum(gsum[:], ps_g[:], axis=mybir.AxisListType.X)
                gb2_sb = pool.tile([E, 1], F32, tag="gb2")
                nc.sync.dma_start(gb2_sb[:], gb2[:, None])
                gmean = pool.tile([E, 1], F32, tag="gmean")
                nc.vector.tensor_scalar_mul(gmean[:], gsum[:], 1.0 / S)
                gown = pool.tile([E, 1], F32, tag="gown")
                nc.vector.tensor_add(gown[:], gmean[:], gb2_sb[:])
                nc.sync.dma_start(ag_in[:], gown[:, 0])

            nc.gpsimd.collective_compute(
                "AllGather",
                mybir.AluOpType.bypass,
                ins=[ag_in[:]],
                outs=[ag_out[:]],
                replica_groups=groups,
            )
            if debug:
                nc.sync.dma_start(dbg_g[:], ag_out[:])

            # g broadcast across partitions [P, B*E]; transposed tiny [E, B]
            g_bc = pool.tile([P, NC * E], F32, tag="gbc")
            nc.sync.dma_start(
                g_bc[:], ag_out.ap()[None, :].broadcast_to([P, NC * E])
            )
            gT_r = pool.tile([E, B], F32R, tag="gT")
            nc.gpsimd.dma_start(gT_r[:], ag_out.rearrange("(b e) -> e b", e=E))

            # identity for the scaled-diag merge trick
            eye = pool.tile([P, P], F32, tag="eye")
            make_identity(nc, eye[:])

            # ---- merged per-sample biases ----
            # b1T[:, mt*8+b] = (g[b] @ eb1s)[mt-tile]      (full value)
            # b2T[:, m*8+b]  = (g[b] @ eb2)[m-tile] / 8    (1/8: summed by RS)
            eb1_r = pool.tile([E, DSL], F32R, tag="eb1")
            nc.sync.dma_start(eb1_r[:], eb1s[:])
            eb2_f = pool.tile([E, H], F32, tag="eb2f")
            nc.sync.dma_start(eb2_f[:], eb2[:])
            eb2_r8 = pool.tile([E, H], F32R, tag="eb2r")
            nc.scalar.activation(eb2_r8[:], eb2_f[:], AF.Copy, scale=1.0 / NC)
            b1t = pool.tile([P, 4 * B], F32, tag="b1t")
            b2t = pool.tile([P, 8 * B], F32, tag="b2t")
            for mt in range(4):
                ps = pp.tile([P, B], F32, tag="tiny")
                nc.tensor.matmul(
                    ps[:],
                    eb1_r[:, mt * P : (mt + 1) * P],
                    gT_r[:],
                    start=True,
                    stop=True,
                )
                nc.vector.tensor_copy(b1t[:, mt * B : (mt + 1) * B], ps[:])
            for m in range(8):
                ps = pp.tile([P, B], F32, tag="tiny")
                nc.tensor.matmul(
                    ps[:],
                    eb2_r8[:, m * P : (m + 1) * P],
                    gT_r[:],
                    start=True,
                    stop=True,
                )
                nc.vector.tensor_copy(b2t[:, m * B : (m + 1) * B], ps[:])

            def make_gdiag(b):
                tiles = []
                for e in range(E):
                    gd = pool.tile([P, P], F32R, tag="gd", bufs=16)
                    nc.scalar.activation(
                        gd[:],
                        eye[:],
                        AF.Copy,
                        scale=g_bc[:, b * E + e : b * E + e + 1],
                    )
                    tiles.append(gd)
                return tiles

            def phase1():
                HF = 256
                for p in range(2):
                    if True:
                        ewA = pool.tile([P, 4 * E * HF], F32R, tag="ew", bufs=3)
                        ewB = pool.tile([P, 4 * E * HF], F32R, tag="ew", bufs=3)
                        for k in range(4):
                            nc.sync.dma_start(
                                ewA[:, k * E * HF : (k + 1) * E * HF],
                                ew1d[p, k],
                            )
                        for k in range(4):
                            nc.sync.dma_start(
                                ewB[:, k * E * HF : (k + 1) * E * HF],
                                ew1d[p, 4 + k],
                            )

                        state = {}

                        def merge1(b):
                            gd = make_gdiag(b)
                            xb = pool.tile([P, 8 * S], F32R, tag="xb", bufs=2)
                            nc.sync.dma_start(
                                xb[:].rearrange("p (k s) -> p k s", k=8),
                                xt_all.rearrange("b (k p) s -> b p k s", p=P)[b],
                            )
                            w1t = pool.tile([P, 8 * HF], F32R, tag="wmt", bufs=4)
                            for k in range(8):
                                ps = pp.tile([P, HF], F32, tag="mm", bufs=3)
                                for e in range(E):
                                    nc.tensor.matmul(
                                        ps[:],
                                        gd[e][:],
                                        (ewA if k < 4 else ewB)[
                                            :,
                                            ((k % 4) * E + e)
                                            * HF : ((k % 4) * E + e + 1)
                                            * HF,
                                        ],
                                        start=(e == 0),
                                        stop=(e == E - 1),
                                    )
                                nc.vector.tensor_copy(
                                    w1t[:, k * HF : (k + 1) * HF], ps[:]
                                )
                            state[b] = (w1t, xb)

                        def gemm1(b):
                            w1t, xb = state.pop(b)
                            if debug and p == 0 and b == 0:
                                nc.sync.dma_start(dbg_w1t[:], w1t[:].bitcast(F32))
                            for m in range(2):
                                mt = p * 2 + m
                                ps = pp.tile([P, S], F32, tag="out", bufs=3)
                                for k in range(8):
                                    nc.tensor.matmul(
                                        ps[:],
                                        w1t[
                                            :, k * HF + m * P : k * HF + (m + 1) * P
                                        ],
                                        xb[:, k * S : (k + 1) * S],
                                        start=(k == 0),
                                        stop=(k == 7),
                                    )
                                y1 = pool.tile([P, S], F32R, tag="y1", bufs=4)
                                nc.scalar.activation(
                                    y1[:],
                                    ps[:],
                                    AF.Relu,
                                    bias=b1t[:, mt * B + b : mt * B + b + 1],
                                )
                                nc.sync.dma_start(y1_dram[b, mt], y1[:])

                        for b in range(B + 1):
                            if b < B:
                                merge1(b)
                            if b >= 1:
                                gemm1(b - 1)

            def phase2(with_rs=True):
                HH = 512
                for p in range(2):
                    rs_in = rs_in0 if p == 0 else rs_in1
                    if True:
                        ewA = pool.tile([P, 2 * E * HH], F32R, tag="ew", bufs=3)
                        ewB = pool.tile([P, 2 * E * HH], F32R, tag="ew", bufs=3)
                        for kt in range(2):
                            nc.sync.dma_start(
                                ewA[:, kt * E * HH : (kt + 1) * E * HH],
                                ew2d[p, kt],
                            )
                        for kt in range(2):
                            nc.sync.dma_start(
                                ewB[:, kt * E * HH : (kt + 1) * E * HH],
                                ew2d[p, 2 + kt],
                            )

                        state2 = {}

                        def merge2(b):
                            gd = make_gdiag(b)
                            yb = pool.tile([P, 4 * S], F32R, tag="wmt", bufs=4)
                            nc.sync.dma_start(
                                yb[:].rearrange("p (k s) -> p k s", k=4),
                                y1_dram.rearrange("b k p s -> b p k s")[b],
                            )
                            w2t = pool.tile([P, 4 * HH], F32R, tag="wmt", bufs=4)
                            for kt in range(4):
                                ps = pp.tile([P, HH], F32, tag="mm", bufs=3)
                                for e in range(E):
                                    nc.tensor.matmul(
                                        ps[:],
                                        gd[e][:],
                                        (ewA if kt < 2 else ewB)[
                                            :,
                                            ((kt % 2) * E + e)
                                            * HH : ((kt % 2) * E + e + 1)
                                            * HH,
                                        ],
                                        start=(e == 0),
                                        stop=(e == E - 1),
                                    )
                                nc.vector.tensor_copy(
                                    w2t[:, kt * HH : (kt + 1) * HH], ps[:]
                                )
                            state2[b] = (w2t, yb)

                        def gemm2(b):
                            w2t, yb = state2.pop(b)
                            for m in range(4):
                                mg = p * 4 + m
                                ps = pp.tile([P, S], F32, tag="out", bufs=3)
                                for kt in range(4):
                                    nc.tensor.matmul(
                                        ps[:],
                                        w2t[
                                            :,
                                            kt * HH + m * P : kt * HH + (m + 1) * P,
                                        ],
                                        yb[:, kt * S : (kt + 1) * S],
                                        start=(kt == 0),
                                        stop=(kt == 3),
                                    )
                                y2 = pool.tile([P, S], F32, tag="y2", bufs=4)
                                nc.scalar.activation(
                                    y2[:],
                                    ps[:],
                                    AF.Identity,
                                    bias=b2t[:, mg * B + b : mg * B + b + 1],
                                )
                                nc.sync.dma_start(rs_in[b, m], y2[:])

                        for b in range(B + 1):
                            if b < B:
                                merge2(b)
                            if b >= 1:
                                gemm2(b - 1)

                    if with_rs:
                        nc.gpsimd.collective_compute(
                            "ReduceScatter",
                            mybir.AluOpType.add,
                            ins=[rs_in.ap().rearrange("b m p s -> (b m p) s")],
                            outs=[(rs_out0 if p == 0 else rs_out1)[:]],
                            replica_groups=groups,
                        )

            if time_loop:
                with tc.For_i(0, time_loop, 1):
                    phase1()
                    phase2(with_rs=False)
                nc.sync.dma_start(y_out[0 : 4 * P], rs_in0.ap()[0])
                nc.sync.dma_start(y_out[4 * P : 8 * P], rs_in1.ap()[0])
            else:
                phase1()
                phase2(with_rs=True)
                if debug:
                    for mt in range(4):
                        nc.sync.dma_start(dbg_y1[mt], y1_dram[0, mt].bitcast(F32))
                    for m in range(4):
                        nc.sync.dma_start(dbg_rs0[m], rs_in0[0, m])
                nc.sync.dma_start(y_out[0 : 4 * P], rs_out0[:])
                nc.sync.dma_start(y_out[4 * P : 8 * P], rs_out1[:])

    nc.compile()
    return nc


def _ew1_dev(a):
    # a: [E, DSL(o), H(i)] -> [2pass, 8k, 128p(i), 8e * 256o]
    a2 = np.ascontiguousarray(np.asarray(a, np.float32).transpose(2, 0, 1))
    a3 = a2.reshape(8, P, E, 2, 256).transpose(3, 0, 1, 2, 4)
    return np.ascontiguousarray(a3.reshape(2, 8, P, E * 256))


def _ew2_dev(c):
    # c: [E, H(h), DSL(d)] -> [2pass, 4kt, 128p(d), 8e * 512h]
    c2 = np.ascontiguousarray(np.asarray(c, np.float32).transpose(2, 0, 1))
    c3 = c2.reshape(4, P, E, 2, 512).transpose(3, 0, 1, 2, 4)
    return np.ascontiguousarray(c3.reshape(2, 4, P, E * 512))


def _shard_inputs(hidden_states, gW1, gb1, gW2, gb2, eW1, eb1, eW2, eb2):
    xt_all = np.ascontiguousarray(
        np.asarray(hidden_states, dtype=np.float32).transpose(1, 2, 0)
    )  # [B, H, S]
    gW1t = np.ascontiguousarray(np.asarray(gW1, np.float32).T)
    gb1t = np.ascontiguousarray(np.asarray(gb1, np.float32).reshape(8, P).T)
    gW2t = np.ascontiguousarray(np.asarray(gW2, np.float32).T)
    gb2 = np.ascontiguousarray(np.asarray(gb2, np.float32))
    eW1 = np.asarray(eW1, np.float32)
    eW2 = np.asarray(eW2, np.float32)
    eb1 = np.asarray(eb1, np.float32)
    eb2 = np.ascontiguousarray(np.asarray(eb2, np.float32))
    in_maps = []
    for j in range(NC):
        sl = slice(j * DSL, (j + 1) * DSL)
        in_maps.append(
            {
                "xt_all": xt_all,
                "xt_own": np.ascontiguousarray(xt_all[j]),
                "gw1t": gW1t,
                "gb1t": gb1t,
                "gw2t": gW2t,
                "gb2": gb2,
                "ew1d": _ew1_dev(eW1[:, sl, :]),
                "ew2d": _ew2_dev(eW2[:, :, sl]),
                "eb1s": np.ascontiguousarray(eb1[:, sl]),
                "eb2": eb2,
            }
        )
    return in_maps


# ---------------- SPMD runner (persistent jit over axon PJRT) -----------

_CACHE = {}


def _build_runner(debug=False, time_loop=0):
    import jax
    from jax.sharding import Mesh, PartitionSpec
    from jax.experimental.shard_map import shard_map
    from concourse import bass2jax

    nc = build_module(debug=debug, time_loop=time_loop)
    bass2jax.install_neuronx_cc_hook()
    partition_name = nc.partition_id_tensor.name if nc.partition_id_tensor else None

    in_names, out_names, out_avals = [], [], []
    for alloc in nc.m.functions[0].allocations:
        if not isinstance(alloc, mybir.MemoryLocationSet):
            continue
        name = alloc.memorylocations[0].name
        if alloc.kind == "ExternalInput":
            if name != partition_name:
                in_names.append(name)
        elif alloc.kind == "ExternalOutput":
            out_avals.append(
                jax.core.ShapedArray(
                    tuple(alloc.tensor_shape), mybir.dt.np(alloc.dtype)
                )
            )
            out_names.append(name)
    n_outs = len(out_names)
    all_in_names = list(in_names) + list(out_names)
    if partition_name is not None:
        all_in_names.append(partition_name)

    def _body(*args):
        operands = list(args)
        if partition_name is not None:
            operands.append(bass2jax.partition_id_tensor())
        return tuple(
            bass2jax._bass_exec_p.bind(
                *operands,
                out_avals=tuple(out_avals),
                in_names=tuple(all_in_names),
                out_names=tuple(out_names),
                lowering_input_output_aliases=(),
                sim_require_finite=True,
                sim_require_nnan=True,
                nc=nc,
            )
        )

    devices = jax.devices()[:NC]
    mesh = Mesh(np.asarray(devices), ("core",))
    n_params = len(in_names)
    sharded = jax.jit(
        shard_map(
            _body,
            mesh=mesh,
            in_specs=(PartitionSpec("core"),) * (n_params + n_outs),
            out_specs=(PartitionSpec("core"),) * n_outs,
            check_rep=False,
        ),
        keep_unused=True,
    )
    zero_shapes = [((NC * a.shape[0], *a.shape[1:]), a.dtype) for a in out_avals]

    def run(in_maps, device_inputs=None, fetch=True):
        if device_inputs is None:
            concat_in = [
                np.concatenate(
                    [np.asarray(in_maps[c][n]) for c in range(NC)], axis=0
                )
                for n in in_names
            ]
            dev_params = [jax.device_put(x) for x in concat_in]
            dev_zeros = [jax.device_put(np.zeros(s, d)) for s, d in zero_shapes]
            device_inputs = (dev_params, dev_zeros)
            jax.block_until_ready(dev_params)
            jax.block_until_ready(dev_zeros)
        dev_params, dev_zeros = device_inputs
        out_arrs = sharded(*dev_params, *dev_zeros)
        jax.block_until_ready(out_arrs)
        if not fetch:
            return None, device_inputs
        results = [
            {
                name: np.asarray(out_arrs[i]).reshape(NC, *out_avals[i].shape)[c]
                for i, name in enumerate(out_names)
            }
            for c in range(NC)
        ]
        return results, device_inputs

    return run


def get_runner(debug=False, time_loop=0):
    key = ("run", debug, time_loop)
    if key not in _CACHE:
        _CACHE[key] = _build_runner(debug=debug, time_loop=time_loop)
    return _CACHE[key]


def kernel(**inputs) -> np.ndarray:
    run = get_runner()
    in_maps = _shard_inputs(**inputs)
    results, _ = run(in_maps)
    # core b's output is y2^T[b] = [H, S]; assemble [S, B, H]
    y2t = np.stack([results[b]["y2t"] for b in range(B)], axis=0)  # [B, H, S]
    return np.ascontiguousarray(y2t.transpose(2, 0, 1)).astype(np.float32)
